# revision 1
# baseline (speedup 1.0000x reference)
"""CondConv2d on 8 Trainium2 NeuronCores — data-parallel over batch N=8.

Per-core (one sample):
  - The attention branch (three global-mean-pooled conv3ds) collapses to a
    linear function of 13 "basis" sums of x: 4 partial totals, edge rows/cols,
    corners, and a constant.  Basis sums are computed with fused
    accumulate-reductions split across the Vector and Scalar engines, the
    (channel x basis) x coefficient contraction runs as 4 tiny fused DVE ops +
    one 64->128-broadcast matmul, then softmax and per-sample weight mixing
    (the static residual conv is fused in: mw = sum_k att_k W_k + conv_w;
    conv bias is added at PSUM eviction).
  - The 3x3 conv runs as 6 accumulating PE matmuls per PSUM tile over a
    130-wide zero-padded layout; contraction 128 = 64 channels (lower
    partitions) + 64 channels of a row-shifted copy (upper partitions),
    pairing taps (-1,w)+(0,w) per matmul.  The row-shifted copy is produced
    by an on-chip SBUF->SBUF DMA so x is read from HBM only once.
"""
import os
import numpy as np

N, C, H, W = 8, 64, 128, 128
K = 4
WP = W + 2                 # padded row width (130)
NELEM = WP * WP + 2        # per-partition x buffer length (16902)
ROWS_PER_TILE = 3          # output rows per PSUM tile (free dim 390 <= 512)
NCHUNKS = 4                # x load chunks

CONV_DT = os.environ.get("KCONV_DT", "fp32r")   # "fp32" | "fp32r" | "bf16"

MM_TAPS = [((-1, -1), (0, -1)), ((-1, 0), (0, 0)), ((-1, 1), (0, 1)),
           ((1, -1), None), ((1, 0), None), ((1, 1), None)]
MM_OFFS = [130 * L[0] + L[1] for L, _ in MM_TAPS]


# ----------------------------------------------------------------------------
# host-side prep
# ----------------------------------------------------------------------------
def _make_cw2(net0_w, net0_b, net1_w, net1_b, net2_w, net2_b):
    """CW2[c, b, k]: logits[k] = sum_{c,b} CW2[c,b,k] * basis[c,b].
    basis: 0=total, 1=row0, 2=row127, 3=col0, 4=col127,
           5..8=corners (00,0W,H0,HW), 9=const 1."""
    cw = np.zeros((C, 10, K), np.float64)
    scale = 1.0 / (C * H * W)
    for w_net, pads in ((net0_w, (0, 0, 0)), (net1_w, (1, 1, 1)), (net2_w, (2, 1, 1))):
        Kk, _, kd, kh, kw = w_net.shape
        pd, ph, pw = pads
        for i in range(kd):
            clo, chi = max(0, i - pd), min(C - 1, C - 1 + i - pd)
            cmask = np.zeros(C)
            cmask[clo:chi + 1] = 1.0
            for j in range(kh):
                hlo, hhi = max(0, j - ph), min(H - 1, H - 1 + j - ph)
                dropA = 0 if hlo == 1 else (127 if hhi == H - 2 else None)
                for l in range(kw):
                    wlo, whi = max(0, l - pw), min(W - 1, W - 1 + l - pw)
                    dropB = 0 if wlo == 1 else (127 if whi == W - 2 else None)
                    v = np.zeros(10)
                    v[0] = 1.0
                    if dropA == 0: v[1] = -1.0
                    if dropA == 127: v[2] = -1.0
                    if dropB == 0: v[3] = -1.0
                    if dropB == 127: v[4] = -1.0
                    if dropA is not None and dropB is not None:
                        v[{(0, 0): 5, (0, 127): 6, (127, 0): 7, (127, 127): 8}[(dropA, dropB)]] = 1.0
                    for k in range(Kk):
                        cw[:, :, k] += w_net[k, 0, i, j, l] * scale * np.outer(cmask, v)
    btot = (net0_b + net1_b + net2_b).astype(np.float64)
    cw[:, 9, :] += btot[None, :] / C
    return np.ascontiguousarray(cw.astype(np.float32))


def _make_bank(Wt):
    """Wt (co, ci, 3, 3) -> (128, 6, 64): [p=ci(lo)/64+ci(hi), mm, co]."""
    bank = np.zeros((128, 6, 64), np.float32)
    for m, (L, Hh) in enumerate(MM_TAPS):
        bank[:64, m, :] = Wt[:, :, 1 + L[0], 1 + L[1]].T
        if Hh is not None:
            bank[64:, m, :] = Wt[:, :, 1 + Hh[0], 1 + Hh[1]].T
    return bank


# ----------------------------------------------------------------------------
# device program
# ----------------------------------------------------------------------------
_NC_CACHE = {}


def _build_nc(conv_dt):
    import concourse.bacc as bacc
    import concourse.tile as tile
    from concourse import mybir

    f32 = mybir.dt.float32
    if conv_dt == "bf16":
        DT = mybir.dt.bfloat16
    elif conv_dt == "fp32r":
        DT = mybir.dt.float32r
    else:
        DT = f32
    WBDT = mybir.dt.bfloat16 if conv_dt == "bf16" else f32
    MWDT = mybir.dt.float32r if conv_dt == "fp32r" else f32
    Alu = mybir.AluOpType
    Ax = mybir.AxisListType
    Act = mybir.ActivationFunctionType

    nc = bacc.Bacc("TRN2", target_bir_lowering=False, debug=False,
                   enable_asserts=False, num_devices=N)
    xin = nc.dram_tensor("xin", [C, H * WP], DT, kind="ExternalInput")
    wbk = nc.dram_tensor("wbanks", [128, 5, 6 * 64], WBDT, kind="ExternalInput")
    cw2 = nc.dram_tensor("cw2", [C, 10, K], f32, kind="ExternalInput")
    cb = nc.dram_tensor("convb", [C, 1], f32, kind="ExternalInput")
    outT = nc.dram_tensor("out", [C, H, W], f32, kind="ExternalOutput")

    span_elems = WP * (H // NCHUNKS)                   # 8320

    with tile.TileContext(nc) as tc:
        with tc.tile_pool(name="singles", bufs=1) as S, \
             tc.tile_pool(name="stage", bufs=4) as STG, \
             tc.tile_pool(name="cpsum", bufs=4, space="PSUM") as PS, \
             tc.tile_pool(name="spsum", bufs=1, space="PSUM") as PS1:

            XL = S.tile([128, NELEM], DT)
            wb_sb = S.tile([128, 5, 6 * 64], WBDT)
            cw2_sb = S.tile([C, 10, K], f32)
            convb_sb = S.tile([C, 1], f32)
            onesrow = S.tile([128, 128], f32)
            onesall = S.tile([C, 128], f32)
            att_sb = S.tile([128, K], f32)
            attbc = S.tile([128, K], f32)
            M10 = S.tile([C, 10], f32)
            P01 = S.tile([C, 1], f32)
            P23 = S.tile([C, 1], f32)
            PART0 = S.tile([C, 1], f32)
            PART1 = S.tile([C, 1], f32)
            PART2 = S.tile([C, 1], f32)
            PART3 = S.tile([C, 1], f32)
            PART4 = S.tile([C, 1], f32)
            PARTS = [PART0, PART1, PART2, PART3, PART4]
            G = S.tile([C, K], f32)
            mw = S.tile([128, 6, 64], MWDT)
            mwb = S.tile([128, 6, 64], DT, name="mwb") if conv_dt == "bf16" else None
            fold = S.tile([C, 2700], f32)
            fold2 = S.tile([C, 2700], f32)
            actout = S.tile([C, 4300], f32)
            actout2 = S.tile([C, 4300], f32)
            rs128 = S.tile([128, 1], f32)

            wpsum = PS1.tile([128, 512], f32)
            psum_b = PS1.tile([128, K], f32)

            XLv = XL.bitcast(f32) if conv_dt == "fp32r" else XL

            # --- constants / border zeroing (DVE, all tiny) ---
            nc.vector.memset(onesrow, 0.0)
            nc.vector.memset(onesall, 1.0)
            nc.vector.memset(M10[:, 9:10], 1.0)
            # borders: host pre-pads the row gaps; only head/tail need zeroing
            nc.vector.memset(XLv[0:64, 0:132], 0.0)
            nc.vector.memset(XLv[0:64, 132 + H * WP:NELEM], 0.0)
            nc.vector.memset(XLv[64:128, 0:2], 0.0)
            nc.vector.memset(XLv[64:128, 2 + H * WP:NELEM], 0.0)

            # --- small input DMAs (scalar/ACT HWDGE ring) ---
            nc.scalar.dma_start(out=wb_sb, in_=wbk[:, :, :])
            nc.scalar.dma_start(out=cw2_sb, in_=cw2[:, :, :])
            nc.scalar.dma_start(out=convb_sb, in_=cb[:, :])

            # --- x load: contiguous chunks; lower (parts 0-63) and row-shifted
            # upper copy (parts 64-127) kept in flight together so the two DMAs
            # cover complementary SBUF ports (full DMA bandwidth)
            for c in range(NCHUNKS):
                a = span_elems * c
                nc.sync.dma_start(out=XL[0:64, 132 + a: 132 + a + span_elems],
                                  in_=xin[:, a: a + span_elems])
                nc.sync.dma_start(out=XL[64:128, 2 + a: 2 + a + span_elems],
                                  in_=xin[:, a: a + span_elems])

            # --- PE warm-up (results discarded; onesrow is all-zero) ---
            for i in range(8):
                nc.tensor.matmul(wpsum[:, 0:128], onesrow, onesrow, start=True, stop=True)

            # --- attention basis sums ---
            # DVE: scalar_tensor_tensor fold (2 streams/cycle) with accum_out;
            # ACT: activation-Identity with accum_out.  (tensor_tensor_reduce
            # is broken on this runtime — do not use.)
            spans = [(0, 4292, "dve", fold), (4292, 4160, "act", actout),
                     (8452, 4160, "dve", fold2), (12612, 2146, "act", actout2),
                     (14758, NELEM - 14758, "dve", fold)]
            # row-0 edge sum can start as soon as chunk 0 lands
            nc.vector.tensor_reduce(out=M10[:, 1:2], in_=XLv[0:64, 132:132 + W],
                                    axis=Ax.X, op=Alu.add)
            for idx, (a, ln, eng, obuf) in enumerate(spans):
                if idx == 3:
                    # chunk-1-dependent small reductions, queued on DVE before
                    # the second big span so the logits pipeline unblocks early
                    nc.vector.tensor_reduce(out=M10[:, 2:3], in_=XLv[0:64, 16642:16642 + W],
                                            axis=Ax.X, op=Alu.add)
                    col0 = XLv[0:64, 132:132 + WP * H].rearrange("p (r w) -> p r w", w=WP)[:, :, 0:1]
                    nc.vector.tensor_reduce(out=M10[:, 3:4], in_=col0, axis=Ax.XY, op=Alu.add)
                    col1 = XLv[0:64, 259:259 + WP * H].rearrange("p (r w) -> p r w", w=WP)[:, :, 0:1]
                    nc.vector.tensor_reduce(out=M10[:, 4:5], in_=col1, axis=Ax.XY, op=Alu.add)
                    # corners {132,259} and {16642,16769} via stride-127 views
                    nc.vector.tensor_copy(
                        out=M10[:, 5:7].rearrange("p (a b) -> p a b", b=1),
                        in_=XLv[0:64, 132:132 + 254].rearrange("p (a b) -> p a b", b=127)[:, :, 0:1])
                    nc.vector.tensor_copy(
                        out=M10[:, 7:9].rearrange("p (a b) -> p a b", b=1),
                        in_=XLv[0:64, 16642:16642 + 254].rearrange("p (a b) -> p a b", b=127)[:, :, 0:1])
                if eng == "dve":
                    h = ln // 2
                    nc.vector.scalar_tensor_tensor(
                        out=obuf[:, :h], in0=XLv[0:64, a:a + h], scalar=1.0,
                        in1=XLv[0:64, a + h:a + ln], op0=Alu.mult, op1=Alu.add,
                        accum_out=PARTS[idx][:, 0:1])
                else:
                    nc.scalar.activation(
                        out=obuf[:, :ln], in_=XLv[0:64, a:a + ln], func=Act.Identity,
                        bias=0.0, scale=1.0, accum_out=PARTS[idx][:, 0:1])

            # fold the 5 span partials into basis column 0 (all on DVE)
            nc.vector.tensor_add(out=P01, in0=PART0, in1=PART1)
            nc.vector.tensor_add(out=P23, in0=PART2, in1=PART3)
            nc.vector.tensor_add(out=P01, in0=P01, in1=PART4)
            nc.vector.tensor_add(out=M10[:, 0:1], in0=P01, in1=P23)

            # per-channel coefficient contraction: G[c,k] = sum_b M10[c,b]*CW2[c,b,k]
            for k in range(K):
                nc.vector.scalar_tensor_tensor(
                    out=actout2[:, 0:10], in0=M10[:, :], scalar=1.0,
                    in1=cw2_sb[:, :, k], op0=Alu.mult, op1=Alu.mult,
                    accum_out=G[:, k:k + 1])

            # keep the PE clock warm into the conv (dummy matmuls on span scratch)
            for i in range(4):
                nc.tensor.matmul(wpsum, onesrow[0:64, :], actout[:, i * 512:(i + 1) * 512],
                                 start=True, stop=True)

            # logits broadcast to all 128 partitions with one matmul
            nc.tensor.matmul(psum_b, onesall, G, start=True, stop=True)
            # softmax per partition (identical everywhere); logits are tiny, so
            # the max-subtraction is unnecessary
            nc.scalar.activation(out=att_sb, in_=psum_b, func=Act.Exp)
            nc.vector.tensor_reduce(out=rs128, in_=att_sb, axis=Ax.X, op=Alu.add)
            nc.vector.reciprocal(out=rs128, in_=rs128)
            nc.vector.tensor_scalar_mul(out=attbc, in0=att_sb, scalar1=rs128)

            # --- weight mixing: mw = conv_bank + sum_k att_k * bank_k ---
            # two m-groups so the first conv matmuls can start while the second
            # half of the mixed weight is still being built
            wbv = wb_sb[:, :, :].rearrange("p b (m c) -> p b m c", m=6)
            for g in (slice(0, 3), slice(3, 6)):
                nc.vector.scalar_tensor_tensor(
                    out=mw[:, g, :], in0=wbv[:, 0, g, :], scalar=attbc[:, 0:1],
                    in1=wbv[:, 4, g, :], op0=Alu.mult, op1=Alu.add)
                for k in range(1, K):
                    tgt = mwb if (k == K - 1 and conv_dt == "bf16") else mw
                    nc.vector.scalar_tensor_tensor(
                        out=tgt[:, g, :], in0=wbv[:, k, g, :],
                        scalar=attbc[:, k:k + 1], in1=mw[:, g, :],
                        op0=Alu.mult, op1=Alu.add)
            lhs_src = mwb if conv_dt == "bf16" else mw

            # --- main conv: 43 PSUM tiles x 6 accumulating matmuls ---
            for ti, r0 in enumerate(range(1, H + 1, ROWS_PER_TILE)):
                nrows = min(ROWS_PER_TILE, H + 1 - r0)
                F = WP * nrows
                pt = PS.tile([64, WP * ROWS_PER_TILE], f32, tag="cps", name=f"cps{ti}")
                pt = pt[:, :F]
                for m in range(6):
                    rhs = XL[:, WP * r0 + MM_OFFS[m] + 1: WP * r0 + MM_OFFS[m] + 1 + F]
                    nc.tensor.matmul(pt, lhs_src[:, m, :], rhs, start=(m == 0), stop=(m == 5))
                st = STG.tile([64, WP * ROWS_PER_TILE], f32, tag="stg", name=f"stg{ti}")
                if ti % 2 == 0:
                    nc.scalar.add(out=st[:, :F], in_=pt, add=convb_sb[:, 0:1])
                else:
                    nc.vector.tensor_scalar_add(out=st[:, :F], in0=pt, scalar1=convb_sb[:, 0:1])
                src = st[:, :F].rearrange("p (r w) -> p r w", w=WP)[:, :, 1:1 + W]
                eng = nc.sync if ti % 2 == 0 else nc.scalar
                eng.dma_start(out=outT[:, r0 - 1:r0 - 1 + nrows, :], in_=src)

    nc.compile()
    return nc


def _get_nc():
    if CONV_DT not in _NC_CACHE:
        _NC_CACHE[CONV_DT] = _build_nc(CONV_DT)
    return _NC_CACHE[CONV_DT]


def _prep_inputs(x, weight, conv_w, conv_b, net0_w, net0_b, net1_w, net1_b,
                 net2_w, net2_b):
    cw2 = _make_cw2(np.asarray(net0_w, np.float32), np.asarray(net0_b, np.float32),
                    np.asarray(net1_w, np.float32), np.asarray(net1_b, np.float32),
                    np.asarray(net2_w, np.float32), np.asarray(net2_b, np.float32))
    banks = np.stack([_make_bank(np.asarray(weight, np.float32)[k]) for k in range(K)]
                     + [_make_bank(np.asarray(conv_w, np.float32))])  # (5,128,6,64)
    banks = np.ascontiguousarray(banks.reshape(5, 128, 6 * 64).transpose(1, 0, 2))
    convb = np.ascontiguousarray(np.asarray(conv_b, np.float32).reshape(C, 1))
    x = np.asarray(x, np.float32)
    xp = np.zeros((N, C, H, WP), np.float32)
    xp[:, :, :, :W] = x
    if CONV_DT == "bf16":
        import ml_dtypes
        xs = xp.astype(ml_dtypes.bfloat16)
        banks = banks.astype(ml_dtypes.bfloat16)
    else:
        xs = xp
    in_maps = []
    for n in range(N):
        in_maps.append({
            "xin": np.ascontiguousarray(xs[n].reshape(C, H * WP)),
            "wbanks": banks,
            "cw2": cw2,
            "convb": convb,
        })
    return in_maps


def _run(inputs, trace=False, **kw):
    from concourse.bass_utils import run_bass_kernel_spmd
    nc = _get_nc()
    in_maps = _prep_inputs(**inputs)
    return run_bass_kernel_spmd(nc, in_maps, core_ids=list(range(N)), trace=trace, **kw)


def kernel(**inputs):
    res = _run(inputs)
    out = np.stack([res.results[n]["out"] for n in range(N)]).astype(np.float32)
    return out



# revision 9
# speedup vs baseline: 1.0594x; 1.0594x over previous
"""CondConv2d on 8 Trainium2 NeuronCores — data-parallel over batch N=8.

v2 design (per core, one sample):
  - Everything in bf16 on the wire: x (2.13MB), weight banks, output.
  - The static residual conv is folded into the expert banks host-side
    (W'_k = W_k + conv_w; since sum_k softmax_k = 1, mixing with
    unnormalized exp(logit) weights and scaling by 1/sum at PSUM
    eviction reproduces conv(x, sum att_k W_k + conv_w)).  conv bias is
    the eviction's add operand.
  - x is loaded lower-copy-first across BOTH hwdge queues (sync+scalar)
    in 4 chunks (last chunk small so the attention tail is short); the
    row-shifted upper copy follows.  Attention basis sums run per chunk
    on DVE+ACT with gpsimd taking the edge/corner reductions.
  - conv: 43 PSUM tiles x 6 accumulating bf16 matmuls (128-contraction
    = 64ch row r-1/r pairs in lower/upper partition halves).
  - Evictions strip the 130-wide padding into a packed bf16 staging
    buffer; one out-DMA per 4 tiles (12 rows, 3KB contiguous/channel).
"""
import os
import numpy as np

N, C, H, W = 8, 64, 128, 128
K = 4
WP = W + 2                 # padded row width (130)
NELEM = WP * WP + 2        # per-partition x buffer length (16902)
ROWS_PER_TILE = 3
HWELEM = H * WP            # 16640

# lower-copy chunks (elements per partition); small tail chunk so the
# final attention reductions are short
CHUNKS = [5200, 5200, 5200, 1040]
CHUNK_OFF = [0, 5200, 10400, 15600]
# upper-copy chunks
UCHUNKS = [4160, 4160, 4160, 4160]
UCHUNK_OFF = [0, 4160, 8320, 12480]

TILES_PER_DMA = 4

MM_TAPS = [((-1, -1), (0, -1)), ((-1, 0), (0, 0)), ((-1, 1), (0, 1)),
           ((1, -1), None), ((1, 0), None), ((1, 1), None)]
MM_OFFS = [130 * L[0] + L[1] for L, _ in MM_TAPS]


# ----------------------------------------------------------------------------
# host-side prep
# ----------------------------------------------------------------------------
def _make_cw2(net0_w, net0_b, net1_w, net1_b, net2_w, net2_b):
    """CW2[c, b, k]: logits[k] = sum_{c,b} CW2[c,b,k] * basis[c,b].
    basis: 0=total, 1=row0, 2=row127, 3=col0, 4=col127,
           5..8=corners (00,0W,H0,HW), 9=const 1."""
    cw = np.zeros((C, 10, K), np.float64)
    scale = 1.0 / (C * H * W)
    for w_net, pads in ((net0_w, (0, 0, 0)), (net1_w, (1, 1, 1)), (net2_w, (2, 1, 1))):
        Kk, _, kd, kh, kw = w_net.shape
        pd, ph, pw = pads
        for i in range(kd):
            clo, chi = max(0, i - pd), min(C - 1, C - 1 + i - pd)
            cmask = np.zeros(C)
            cmask[clo:chi + 1] = 1.0
            for j in range(kh):
                hlo, hhi = max(0, j - ph), min(H - 1, H - 1 + j - ph)
                dropA = 0 if hlo == 1 else (127 if hhi == H - 2 else None)
                for l in range(kw):
                    wlo, whi = max(0, l - pw), min(W - 1, W - 1 + l - pw)
                    dropB = 0 if wlo == 1 else (127 if whi == W - 2 else None)
                    v = np.zeros(10)
                    v[0] = 1.0
                    if dropA == 0: v[1] = -1.0
                    if dropA == 127: v[2] = -1.0
                    if dropB == 0: v[3] = -1.0
                    if dropB == 127: v[4] = -1.0
                    if dropA is not None and dropB is not None:
                        v[{(0, 0): 5, (0, 127): 6, (127, 0): 7, (127, 127): 8}[(dropA, dropB)]] = 1.0
                    for k in range(Kk):
                        cw[:, :, k] += w_net[k, 0, i, j, l] * scale * np.outer(cmask, v)
    btot = (net0_b + net1_b + net2_b).astype(np.float64)
    cw[:, 9, :] += btot[None, :] / C
    return np.ascontiguousarray(cw.astype(np.float32))


def _make_bank(Wt):
    """Wt (co, ci, 3, 3) -> (128, 6, 64): [p=ci(lo)/64+ci(hi), mm, co]."""
    bank = np.zeros((128, 6, 64), np.float32)
    for m, (L, Hh) in enumerate(MM_TAPS):
        bank[:64, m, :] = Wt[:, :, 1 + L[0], 1 + L[1]].T
        if Hh is not None:
            bank[64:, m, :] = Wt[:, :, 1 + Hh[0], 1 + Hh[1]].T
    return bank


# ----------------------------------------------------------------------------
# device program
# ----------------------------------------------------------------------------
_NC_CACHE = {}


def _build_nc():
    import concourse.bacc as bacc
    import concourse.tile as tile
    from concourse import mybir

    f32 = mybir.dt.float32
    bf16 = mybir.dt.bfloat16
    Alu = mybir.AluOpType
    Ax = mybir.AxisListType
    Act = mybir.ActivationFunctionType

    nc = bacc.Bacc("TRN2", target_bir_lowering=False, debug=False,
                   enable_asserts=False, num_devices=N)
    xin = nc.dram_tensor("xin", [C, HWELEM], bf16, kind="ExternalInput")
    wbk = nc.dram_tensor("wbanks", [128, K, 6 * 64], bf16, kind="ExternalInput")
    sm = nc.dram_tensor("smalls", [C, 41], f32, kind="ExternalInput")
    outT = nc.dram_tensor("out", [C, H, W], bf16, kind="ExternalOutput")

    with tile.TileContext(nc) as tc:
        with tc.tile_pool(name="singles", bufs=1) as S, \
             tc.tile_pool(name="stage", bufs=2) as STG, \
             tc.tile_pool(name="cpsum", bufs=4, space="PSUM") as PS, \
             tc.tile_pool(name="spsum", bufs=1, space="PSUM") as PS1:

            XL = S.tile([128, NELEM], bf16)
            wb_sb = S.tile([128, K, 6 * 64], bf16)
            sm_sb = S.tile([C, 41], f32)
            zrow = S.tile([128, 128], bf16)       # zeros, warmup lhs
            onesall = S.tile([C, 128], f32)       # ones, logits broadcast
            att_sb = S.tile([128, K], f32)        # exp(logits)
            M10 = S.tile([C, 10], f32)
            PARTS = S.tile([C, 8], f32)
            G = S.tile([C, K], f32)
            mw = S.tile([128, 6, 64], f32)
            mwb = S.tile([128, 6, 64], bf16)
            scr_d = S.tile([C, 2600], bf16)       # DVE fold scratch
            scr_a = S.tile([C, 2600], bf16)       # ACT span scratch
            scr_g = S.tile([C, 16], f32)          # G contraction scratch
            scr_gp = S.tile([C, 128], f32)        # edge-sum scratch
            rs_sum = S.tile([128, 1], f32)
            rs_inv = S.tile([128, 1], f32)

            wpsum = PS1.tile([128, 512], f32)
            psum_b = PS1.tile([128, K], f32)

            cw2v = sm_sb[:, 0:40].rearrange("p (b k) -> p b k", k=K)
            convb = sm_sb[:, 40:41]

            # --- constants / border zeroing ---
            nc.vector.memset(zrow, 0.0)
            nc.vector.memset(onesall, 1.0)
            nc.vector.memset(M10[:, 9:10], 1.0)
            nc.vector.memset(XL[0:64, 0:132], 0.0)
            nc.vector.memset(XL[0:64, 132 + HWELEM:NELEM], 0.0)
            nc.vector.memset(XL[64:128, 0:2], 0.0)
            nc.vector.memset(XL[64:128, 2 + HWELEM:NELEM], 0.0)

            # --- input DMAs ---
            # sync queue: lower chunks 0,1 then wbanks then upper 0,2
            # scalar queue: smalls, lower chunks 2,3, upper 1,3
            nc.scalar.dma_start(out=sm_sb, in_=sm[:, :])
            for c in (0, 1):
                a = CHUNK_OFF[c]
                nc.sync.dma_start(out=XL[0:64, 132 + a:132 + a + CHUNKS[c]],
                                  in_=xin[:, a:a + CHUNKS[c]])
            for c in (2, 3):
                a = CHUNK_OFF[c]
                nc.scalar.dma_start(out=XL[0:64, 132 + a:132 + a + CHUNKS[c]],
                                    in_=xin[:, a:a + CHUNKS[c]])
            nc.sync.dma_start(out=wb_sb, in_=wbk[:, :, :])
            for c, eng in ((0, nc.sync), (1, nc.scalar), (2, nc.sync), (3, nc.scalar)):
                a = UCHUNK_OFF[c]
                eng.dma_start(out=XL[64:128, 2 + a:2 + a + UCHUNKS[c]],
                              in_=xin[:, a:a + UCHUNKS[c]])

            # --- PE warm-up (results discarded; zrow is all-zero) ---
            for i in range(8):
                nc.tensor.matmul(wpsum[:, 0:128], zrow, zrow, start=True, stop=True)

            # --- attention basis sums, per lower chunk ---
            # DVE: fold (2 streams) over ~62%; ACT: plain accum over ~38%
            for c in range(4):
                a = 132 + CHUNK_OFF[c]
                ln = CHUNKS[c]
                dl = (ln * 5 // 8) & ~1          # DVE share (even)
                h = dl // 2
                nc.vector.scalar_tensor_tensor(
                    out=scr_d[:, :h], in0=XL[0:64, a:a + h], scalar=1.0,
                    in1=XL[0:64, a + h:a + dl], op0=Alu.mult, op1=Alu.add,
                    accum_out=PARTS[:, c:c + 1])
                nc.scalar.activation(
                    out=scr_a[:, :ln - dl], in_=XL[0:64, a + dl:a + ln],
                    func=Act.Identity, bias=0.0, scale=1.0,
                    accum_out=PARTS[:, 4 + c:5 + c])
                # keep the PE clock ramped while the load streams
                nc.tensor.matmul(wpsum[:, 0:512], zrow[0:64, :],
                                 scr_d[0:64, 0:512], start=True, stop=True)

            # --- edge/corner sums (lower copy): rows on DVE, cols on ACT ---
            nc.vector.tensor_reduce(out=M10[:, 1:2], in_=XL[0:64, 132:132 + W],
                                    axis=Ax.X, op=Alu.add)
            nc.vector.tensor_reduce(out=M10[:, 2:3], in_=XL[0:64, 16642:16642 + W],
                                    axis=Ax.X, op=Alu.add)
            col0 = XL[0:64, 132:132 + HWELEM].rearrange("p (r w) -> p r w", w=WP)[:, :, 0:1]
            nc.scalar.activation(out=scr_gp[:, 0:128].rearrange("p (a b) -> p a b", b=1),
                                 in_=col0, func=Act.Identity, bias=0.0, scale=1.0,
                                 accum_out=M10[:, 3:4])
            col1 = XL[0:64, 259:259 + HWELEM].rearrange("p (r w) -> p r w", w=WP)[:, :, 0:1]
            nc.scalar.activation(out=scr_gp[:, 0:128].rearrange("p (a b) -> p a b", b=1),
                                 in_=col1, func=Act.Identity, bias=0.0, scale=1.0,
                                 accum_out=M10[:, 4:5])
            nc.vector.tensor_copy(
                out=M10[:, 5:7].rearrange("p (a b) -> p a b", b=1),
                in_=XL[0:64, 132:132 + 254].rearrange("p (a b) -> p a b", b=127)[:, :, 0:1])
            nc.vector.tensor_copy(
                out=M10[:, 7:9].rearrange("p (a b) -> p a b", b=1),
                in_=XL[0:64, 16642:16642 + 254].rearrange("p (a b) -> p a b", b=127)[:, :, 0:1])

            # fold span partials into basis column 0
            nc.vector.tensor_reduce(out=M10[:, 0:1], in_=PARTS, axis=Ax.X, op=Alu.add)

            # per-channel coefficient contraction: G[c,k] = sum_b M10[c,b]*CW2[c,b,k]
            for k in range(K):
                nc.vector.scalar_tensor_tensor(
                    out=scr_g[:, 0:10], in0=M10[:, :], scalar=1.0,
                    in1=cw2v[:, :, k], op0=Alu.mult, op1=Alu.mult,
                    accum_out=G[:, k:k + 1])

            # logits broadcast to all 128 partitions; exp + sum in one ACT op
            nc.tensor.matmul(psum_b, onesall, G, start=True, stop=True)
            nc.scalar.activation(out=att_sb, in_=psum_b, func=Act.Exp,
                                 accum_out=rs_sum)
            nc.vector.reciprocal(out=rs_inv, in_=rs_sum)

            # --- weight mixing: mw = sum_k e_k * bank'_k  (banks have conv_w
            # folded in; 1/sum applied at eviction) ---
            wbv = wb_sb[:, :, :].rearrange("p k (m c) -> p k m c", m=6)
            for g in (slice(0, 3), slice(3, 6)):
                nc.vector.tensor_scalar_mul(
                    out=mw[:, g, :], in0=wbv[:, 0, g, :], scalar1=att_sb[:, 0:1])
                for k in range(1, K):
                    tgt = mwb if k == K - 1 else mw
                    nc.vector.scalar_tensor_tensor(
                        out=tgt[:, g, :], in0=wbv[:, k, g, :],
                        scalar=att_sb[:, k:k + 1], in1=mw[:, g, :],
                        op0=Alu.mult, op1=Alu.add)

            # --- main conv: 43 PSUM tiles x 6 accumulating matmuls ---
            rs64 = rs_inv[0:64, :]
            ntiles = (H + ROWS_PER_TILE - 1) // ROWS_PER_TILE
            stg = None
            for ti in range(ntiles):
                r0 = 1 + ROWS_PER_TILE * ti
                nrows = min(ROWS_PER_TILE, H + 1 - r0)
                F = WP * nrows
                pt = PS.tile([64, WP * ROWS_PER_TILE], f32, tag="cps", name=f"cps{ti}")
                for m in range(6):
                    rhs = XL[:, WP * r0 + MM_OFFS[m] + 1: WP * r0 + MM_OFFS[m] + 1 + F]
                    nc.tensor.matmul(pt[:, :F], mwb[:, m, :], rhs,
                                     start=(m == 0), stop=(m == 5))
                gi = ti % TILES_PER_DMA
                if gi == 0:
                    stg = STG.tile([64, TILES_PER_DMA * ROWS_PER_TILE * W], bf16,
                                   tag="stg", name=f"stg{ti // TILES_PER_DMA}")
                ptv = pt[:, :F].rearrange("p (r w) -> p r w", w=WP)[:, :, 1:1 + W]
                dst = stg[:, gi * ROWS_PER_TILE * W: gi * ROWS_PER_TILE * W + nrows * W]
                dstv = dst.rearrange("p (r w) -> p r w", w=W)
                if ti % 2 == 0:
                    nc.vector.tensor_scalar(out=dstv, in0=ptv, scalar1=rs64,
                                            scalar2=convb, op0=Alu.mult, op1=Alu.add)
                else:
                    nc.scalar.activation(out=dstv, in_=ptv, func=Act.Identity,
                                         scale=rs64, bias=convb)
                if gi == TILES_PER_DMA - 1 or ti == ntiles - 1:
                    g0row = (ti - gi) * ROWS_PER_TILE
                    grows = min(H - g0row, (gi + 1) * ROWS_PER_TILE)
                    src = stg[:, :grows * W].rearrange("p (r w) -> p r w", w=W)
                    nc.sync.dma_start(out=outT[:, g0row:g0row + grows, :], in_=src)

    nc.compile()
    return nc


def _get_nc():
    if "nc" not in _NC_CACHE:
        _NC_CACHE["nc"] = _build_nc()
    return _NC_CACHE["nc"]


def _prep_inputs(x, weight, conv_w, conv_b, net0_w, net0_b, net1_w, net1_b,
                 net2_w, net2_b):
    import ml_dtypes
    cw2 = _make_cw2(np.asarray(net0_w, np.float32), np.asarray(net0_b, np.float32),
                    np.asarray(net1_w, np.float32), np.asarray(net1_b, np.float32),
                    np.asarray(net2_w, np.float32), np.asarray(net2_b, np.float32))
    wsum = np.asarray(weight, np.float32) + np.asarray(conv_w, np.float32)[None]
    banks = np.stack([_make_bank(wsum[k]) for k in range(K)])  # (K,128,6,64)
    banks = np.ascontiguousarray(
        banks.reshape(K, 128, 6 * 64).transpose(1, 0, 2)).astype(ml_dtypes.bfloat16)
    smalls = np.concatenate(
        [cw2.reshape(C, 40), np.asarray(conv_b, np.float32).reshape(C, 1)],
        axis=1).astype(np.float32)
    smalls = np.ascontiguousarray(smalls)
    x = np.asarray(x, np.float32)
    xp = np.zeros((N, C, H, WP), np.float32)
    xp[:, :, :, :W] = x
    xs = xp.astype(ml_dtypes.bfloat16)
    in_maps = []
    for n in range(N):
        in_maps.append({
            "xin": np.ascontiguousarray(xs[n].reshape(C, HWELEM)),
            "wbanks": banks,
            "smalls": smalls,
        })
    return in_maps


def _run(inputs, trace=False, **kw):
    from concourse.bass_utils import run_bass_kernel_spmd
    nc = _get_nc()
    in_maps = _prep_inputs(**inputs)
    return run_bass_kernel_spmd(nc, in_maps, core_ids=list(range(N)), trace=trace, **kw)


def kernel(**inputs):
    res = _run(inputs)
    out = np.stack([np.asarray(res.results[n]["out"]) for n in range(N)]).astype(np.float32)
    return out


# revision 13
# speedup vs baseline: 1.2007x; 1.1333x over previous
"""CondConv2d on 8 Trainium2 NeuronCores — data-parallel over batch N=8.

v3 design (per core, one sample):
  - bf16 end-to-end on the wire; static residual conv folded into the
    expert banks host-side (W'_k = W_k + conv_w), softmax handled by
    normalizing exp(logits) before the weight mix.
  - Dual-plane conv: the PE's full 128 output columns = 64 channels x 2
    output-row planes.  Window (a, c) streams x rows [a, a+2] at column
    offset c; plane A (cols 0-63) computes out rows a+q from taps
    (0,c) [lower ci] + (+1,c) [row-shifted upper ci]; plane B (cols
    64-127) computes out rows a+q+1 from tap (-1,c) [lower ci].  All 9
    taps in 3 matmuls per 3-row tile (vs 6 for the single-plane form).
  - Eviction adds the two planes (plane B of tile j-1 supplies row 3j)
    plus conv bias, strips the 130-wide padding, writes packed bf16
    staging; one out-DMA per 4 tiles.
  - x lower copy is split across both hwdge queues and lands first
    (gates the attention sums); weight banks follow on sync; the
    row-shifted upper copy streams last in 16-row chunks, staying ahead
    of the conv's consumption.
"""
import os
import numpy as np

N, C, H, W = 8, 64, 128, 128
K = 4
WP = W + 2                 # padded row width (130)
NELEM = WP * WP + 2        # per-partition x buffer length (16902)
RPT = 3                    # output rows per PSUM tile
HWELEM = H * WP            # 16640
F3 = WP * RPT              # stream/psum free size (390)

# lower-copy chunks (elements per partition): sync gets 80 rows,
# scalar 48 (its queue starts later); the last chunk is small so the
# attention tail is short
L_SYNC = [(0, 5200), (5200, 5200)]                # rows 0-79
L_SCAL = [(10400, 4160), (14560, 2080)]           # rows 80-127
# upper-copy chunks (16 rows each), alternated so the earliest-needed
# rows land first on the less-loaded queue
U_SCAL = [(0, 2080), (2080, 2080), (8320, 2080), (10400, 2080)]
U_SYNC = [(4160, 2080), (6240, 2080), (12480, 2080), (14560, 2080)]

TILES_PER_DMA = 4


# ----------------------------------------------------------------------------
# host-side prep
# ----------------------------------------------------------------------------
def _make_cw2(net0_w, net0_b, net1_w, net1_b, net2_w, net2_b):
    """CW2[c, b, k]: logits[k] = sum_{c,b} CW2[c,b,k] * basis[c,b].
    basis: 0=total, 1=row0, 2=row127, 3=col0, 4=col127,
           5..8=corners (00,0W,H0,HW), 9=const 1."""
    cw = np.zeros((C, 10, K), np.float64)
    scale = 1.0 / (C * H * W)
    for w_net, pads in ((net0_w, (0, 0, 0)), (net1_w, (1, 1, 1)), (net2_w, (2, 1, 1))):
        Kk, _, kd, kh, kw = w_net.shape
        pd, ph, pw = pads
        for i in range(kd):
            clo, chi = max(0, i - pd), min(C - 1, C - 1 + i - pd)
            cmask = np.zeros(C)
            cmask[clo:chi + 1] = 1.0
            for j in range(kh):
                hlo, hhi = max(0, j - ph), min(H - 1, H - 1 + j - ph)
                dropA = 0 if hlo == 1 else (127 if hhi == H - 2 else None)
                for l in range(kw):
                    wlo, whi = max(0, l - pw), min(W - 1, W - 1 + l - pw)
                    dropB = 0 if wlo == 1 else (127 if whi == W - 2 else None)
                    v = np.zeros(10)
                    v[0] = 1.0
                    if dropA == 0: v[1] = -1.0
                    if dropA == 127: v[2] = -1.0
                    if dropB == 0: v[3] = -1.0
                    if dropB == 127: v[4] = -1.0
                    if dropA is not None and dropB is not None:
                        v[{(0, 0): 5, (0, 127): 6, (127, 0): 7, (127, 127): 8}[(dropA, dropB)]] = 1.0
                    for k in range(Kk):
                        cw[:, :, k] += w_net[k, 0, i, j, l] * scale * np.outer(cmask, v)
    btot = (net0_b + net1_b + net2_b).astype(np.float64)
    cw[:, 9, :] += btot[None, :] / C
    return np.ascontiguousarray(cw.astype(np.float32))


def _make_bank3(Wt):
    """Wt (co, ci, 3, 3) -> (128, 3, 128) dual-plane stationary layout.
    Window w (col offset c=w-1): rows 0-63 = lower ci, rows 64-127 =
    row-shifted upper ci; cols 0-63 = plane A (out row a+q), cols
    64-127 = plane B (out row a+q+1)."""
    bank = np.zeros((128, 3, 128), np.float32)
    for w in range(3):
        bank[:64, w, 0:64] = Wt[:, :, 1, w].T     # A: tap (0, c)
        bank[64:, w, 0:64] = Wt[:, :, 2, w].T     # A: tap (+1, c)
        bank[:64, w, 64:128] = Wt[:, :, 0, w].T   # B: tap (-1, c)
    return bank


# ----------------------------------------------------------------------------
# device program
# ----------------------------------------------------------------------------
_NC_CACHE = {}


def _build_nc():
    import concourse.bacc as bacc
    import concourse.tile as tile
    from concourse import mybir

    f32 = mybir.dt.float32
    bf16 = mybir.dt.bfloat16
    Alu = mybir.AluOpType
    Ax = mybir.AxisListType
    Act = mybir.ActivationFunctionType

    nc = bacc.Bacc("TRN2", target_bir_lowering=False, debug=False,
                   enable_asserts=False, num_devices=N)
    xin = nc.dram_tensor("xin", [C, HWELEM], bf16, kind="ExternalInput")
    # wbs0: mix group 0 banks [128, K, 192] + smalls (cw2 40 + convb 1 on
    # partitions 0-63 in the last 48 cols); wbs1: mix group 1 banks
    wbs0 = nc.dram_tensor("wbs0", [128, K * 192 + 48], bf16, kind="ExternalInput")
    wbs1 = nc.dram_tensor("wbs1", [128, K * 192], bf16, kind="ExternalInput")
    outT = nc.dram_tensor("out", [C, H, W], bf16, kind="ExternalOutput")

    with tile.TileContext(nc) as tc:
        with tc.tile_pool(name="singles", bufs=1) as S, \
             tc.tile_pool(name="stage", bufs=2) as STG, \
             tc.tile_pool(name="bplane", bufs=3) as BP, \
             tc.tile_pool(name="cpsum", bufs=4, space="PSUM") as PS, \
             tc.tile_pool(name="spsum", bufs=1, space="PSUM") as PS1:

            XL = S.tile([128, NELEM], bf16)
            wb0_sb = S.tile([128, K * 192 + 48], bf16)
            wb1_sb = S.tile([128, K * 192], bf16)
            zrow = S.tile([128, 128], bf16)       # zeros, warmup lhs
            onesall = S.tile([C, 128], f32)       # ones, logits broadcast
            att_sb = S.tile([128, K], f32)        # exp(logits)
            attn = S.tile([128, K], f32)          # normalized attention
            M10 = S.tile([C, 10], f32)
            PARTS = S.tile([C, 8], f32)
            G = S.tile([C, K], f32)
            convb = S.tile([C, 1], f32)
            mw = S.tile([128, 3, 128], f32)
            mwb = S.tile([128, 3, 128], bf16)
            scr_d = S.tile([C, 2600], bf16)       # DVE fold scratch
            scr_a = S.tile([C, 2600], bf16)       # ACT span scratch
            scr_g = S.tile([C, 16], f32)          # G contraction scratch
            scr_c = S.tile([C, 128], f32)         # col-sum scratch
            rs_sum = S.tile([128, 1], f32)
            rs_inv = S.tile([128, 1], f32)

            wpsum = PS1.tile([128, 512], f32)
            psum_b = PS1.tile([128, K], f32)

            cw2v = wb0_sb[0:64, K * 192:K * 192 + 40].rearrange(
                "p (b k) -> p b k", k=K)

            # --- constants / border zeroing ---
            nc.vector.memset(zrow, 0.0)
            nc.vector.memset(onesall, 1.0)
            nc.vector.memset(M10[:, 9:10], 1.0)
            nc.vector.memset(XL[0:64, 0:132], 0.0)
            nc.vector.memset(XL[0:64, 132 + HWELEM:NELEM], 0.0)
            nc.vector.memset(XL[64:128, 0:2], 0.0)
            nc.vector.memset(XL[64:128, 2 + HWELEM:NELEM], 0.0)

            # --- input DMAs (queue order == issue order == emission order) ---
            for a, ln in L_SYNC:
                nc.sync.dma_start(out=XL[0:64, 132 + a:132 + a + ln],
                                  in_=xin[:, a:a + ln])
            for a, ln in L_SCAL:
                nc.scalar.dma_start(out=XL[0:64, 132 + a:132 + a + ln],
                                    in_=xin[:, a:a + ln])
            nc.sync.dma_start(out=wb0_sb, in_=wbs0[:, :])
            nc.sync.dma_start(out=wb1_sb, in_=wbs1[:, :])
            for (asc, lsc), (asy, lsy) in zip(U_SCAL, U_SYNC):
                nc.scalar.dma_start(out=XL[64:128, 2 + asc:2 + asc + lsc],
                                    in_=xin[:, asc:asc + lsc])
                nc.sync.dma_start(out=XL[64:128, 2 + asy:2 + asy + lsy],
                                  in_=xin[:, asy:asy + lsy])

            # --- PE warm-up (results discarded; zrow is all-zero) ---
            for i in range(8):
                nc.tensor.matmul(wpsum[:, 0:128], zrow, zrow, start=True, stop=True)

            # --- attention basis sums, per lower chunk ---
            lchunks = L_SYNC + L_SCAL
            for c, (a0, ln) in enumerate(lchunks):
                a = 132 + a0
                dl = (ln * 5 // 8) & ~1          # DVE share (even)
                h = dl // 2
                nc.vector.scalar_tensor_tensor(
                    out=scr_d[:, :h], in0=XL[0:64, a:a + h], scalar=1.0,
                    in1=XL[0:64, a + h:a + dl], op0=Alu.mult, op1=Alu.add,
                    accum_out=PARTS[:, c:c + 1])
                nc.scalar.activation(
                    out=scr_a[:, :ln - dl], in_=XL[0:64, a + dl:a + ln],
                    func=Act.Identity, bias=0.0, scale=1.0,
                    accum_out=PARTS[:, 4 + c:5 + c])
                nc.tensor.matmul(wpsum[:, 0:512], zrow[0:64, :],
                                 scr_d[0:64, 0:512], start=True, stop=True)

            # --- edge/corner sums (lower copy): rows+corners on DVE, cols on ACT ---
            nc.vector.tensor_reduce(out=M10[:, 1:2], in_=XL[0:64, 132:132 + W],
                                    axis=Ax.X, op=Alu.add)
            nc.vector.tensor_reduce(out=M10[:, 2:3], in_=XL[0:64, 16642:16642 + W],
                                    axis=Ax.X, op=Alu.add)
            col0 = XL[0:64, 132:132 + HWELEM].rearrange("p (r w) -> p r w", w=WP)[:, :, 0:1]
            nc.scalar.activation(out=scr_c[:, 0:128].rearrange("p (a b) -> p a b", b=1),
                                 in_=col0, func=Act.Identity, bias=0.0, scale=1.0,
                                 accum_out=M10[:, 3:4])
            col1 = XL[0:64, 259:259 + HWELEM].rearrange("p (r w) -> p r w", w=WP)[:, :, 0:1]
            nc.scalar.activation(out=scr_c[:, 0:128].rearrange("p (a b) -> p a b", b=1),
                                 in_=col1, func=Act.Identity, bias=0.0, scale=1.0,
                                 accum_out=M10[:, 4:5])
            nc.vector.tensor_copy(
                out=M10[:, 5:7].rearrange("p (a b) -> p a b", b=1),
                in_=XL[0:64, 132:132 + 254].rearrange("p (a b) -> p a b", b=127)[:, :, 0:1])
            nc.vector.tensor_copy(
                out=M10[:, 7:9].rearrange("p (a b) -> p a b", b=1),
                in_=XL[0:64, 16642:16642 + 254].rearrange("p (a b) -> p a b", b=127)[:, :, 0:1])

            # copy conv bias (bf16, embedded in wbs0) to fp32
            nc.vector.tensor_copy(out=convb,
                                  in_=wb0_sb[0:64, K * 192 + 40:K * 192 + 41])

            # fold span partials into basis column 0
            nc.vector.tensor_reduce(out=M10[:, 0:1], in_=PARTS, axis=Ax.X, op=Alu.add)

            # per-channel coefficient contraction: G[c,k] = sum_b M10[c,b]*CW2[c,b,k]
            for k in range(K):
                nc.vector.scalar_tensor_tensor(
                    out=scr_g[:, 0:10], in0=M10[:, :], scalar=1.0,
                    in1=cw2v[:, :, k], op0=Alu.mult, op1=Alu.mult,
                    accum_out=G[:, k:k + 1])

            # logits broadcast; exp + sum in one ACT op; normalize on DVE
            nc.tensor.matmul(psum_b, onesall, G, start=True, stop=True)
            nc.scalar.activation(out=att_sb, in_=psum_b, func=Act.Exp,
                                 accum_out=rs_sum)
            nc.vector.reciprocal(out=rs_inv, in_=rs_sum)
            nc.vector.tensor_scalar_mul(out=attn, in0=att_sb, scalar1=rs_inv)

            # --- weight mixing: mw = sum_k attn_k * bank'_k ---
            wv0 = wb0_sb[:, 0:K * 192].rearrange("p (k c) -> p k c", k=K)
            wv1 = wb1_sb[:, :].rearrange("p (k c) -> p k c", k=K)
            mwf = mw.rearrange("p a b -> p (a b)")
            mbf = mwb.rearrange("p a b -> p (a b)")
            for g, wv in ((0, wv0), (1, wv1)):
                sl = slice(g * 192, g * 192 + 192)
                nc.vector.tensor_scalar_mul(
                    out=mwf[:, sl], in0=wv[:, 0, 0:192], scalar1=attn[:, 0:1])
                for k in range(1, K):
                    tgt = mbf if k == K - 1 else mwf
                    nc.vector.scalar_tensor_tensor(
                        out=tgt[:, sl], in0=wv[:, k, 0:192],
                        scalar=attn[:, k:k + 1], in1=mwf[:, sl],
                        op0=Alu.mult, op1=Alu.add)

            # --- main conv: 43 dual-plane PSUM tiles x 3 matmuls ---
            # Eviction: ACT copies plane B (PSUM parts 64-127) into bplane_j
            # aligned to output rows (bplane_j[q] = B-contribution of out row
            # 3j+q, i.e. B_j rows 0,1 -> bplane_j[1:3], B_j row 2 ->
            # bplane_{j+1}[0]); DVE then does ONE STT per tile:
            # stg = (A-plane + convb) + bplane.
            ntiles = (H + RPT - 1) // RPT
            stg = None
            bplanes = {0: BP.tile([64, RPT, W], f32, tag="bp", name="bp0")}
            nc.vector.memset(bplanes[0][:, 0:1, :], 0.0)
            for j in range(ntiles):
                a = RPT * j
                pt = PS.tile([128, F3], f32, tag="cps", name=f"cps{j}")
                for w in range(3):
                    o = 131 + WP * a + (w - 1)
                    nc.tensor.matmul(pt, mwb[:, w, :], XL[:, o:o + F3],
                                     start=(w == 0), stop=(w == 2))
                nrows = min(RPT, H - a)
                gi = j % TILES_PER_DMA
                if gi == 0:
                    stg = STG.tile([64, TILES_PER_DMA * RPT * W], bf16,
                                   tag="stg", name=f"stg{j // TILES_PER_DMA}")
                dst = stg[:, gi * RPT * W: gi * RPT * W + nrows * W].rearrange(
                    "p (r w) -> p r w", w=W)
                ptA = pt[0:64, :].rearrange("p (r w) -> p r w", w=WP)[:, :, 1:1 + W]
                ptB = pt[64:128, :].rearrange("p (r w) -> p r w", w=WP)[:, :, 1:1 + W]
                # ACT: B_j rows 0..nrows-2 -> bplane_j[1:nrows]
                nc.scalar.activation(out=bplanes[j][:, 1:nrows, :],
                                     in_=ptB[:, 0:nrows - 1], func=Act.Identity,
                                     bias=0.0, scale=1.0)
                if j < ntiles - 1:
                    # ACT: B_j row 2 -> bplane_{j+1}[0]
                    bplanes[j + 1] = BP.tile([64, RPT, W], f32, tag="bp",
                                             name=f"bp{j + 1}")
                    nc.scalar.activation(out=bplanes[j + 1][:, 0:1, :],
                                         in_=ptB[:, 2:3],
                                         func=Act.Identity, bias=0.0, scale=1.0)
                # DVE: stg = (A + convb) + bplane
                nc.vector.scalar_tensor_tensor(
                    out=dst, in0=ptA[:, 0:nrows], scalar=convb,
                    in1=bplanes[j][:, 0:nrows, :], op0=Alu.add, op1=Alu.add)
                if gi == TILES_PER_DMA - 1 or j == ntiles - 1:
                    g0row = (j - gi) * RPT
                    grows = min(H - g0row, (gi + 1) * RPT)
                    src = stg[:, :grows * W].rearrange("p (r w) -> p r w", w=W)
                    nc.sync.dma_start(out=outT[:, g0row:g0row + grows, :], in_=src)

    nc.compile()
    return nc


def _get_nc():
    if "nc" not in _NC_CACHE:
        _NC_CACHE["nc"] = _build_nc()
    return _NC_CACHE["nc"]


def _prep_inputs(x, weight, conv_w, conv_b, net0_w, net0_b, net1_w, net1_b,
                 net2_w, net2_b):
    import ml_dtypes
    cw2 = _make_cw2(np.asarray(net0_w, np.float32), np.asarray(net0_b, np.float32),
                    np.asarray(net1_w, np.float32), np.asarray(net1_b, np.float32),
                    np.asarray(net2_w, np.float32), np.asarray(net2_b, np.float32))
    wsum = np.asarray(weight, np.float32) + np.asarray(conv_w, np.float32)[None]
    banks = np.stack([_make_bank3(wsum[k]) for k in range(K)])  # (K,128,3,128)
    # mix groups: g0 = first 192 cols of each bank's (3,128) flattened? No:
    # group by window halves: flatten (3,128)->384, split 192+192
    bf = banks.reshape(K, 128, 384)
    wbs0 = np.zeros((128, K * 192 + 48), np.float32)
    wbs1 = np.zeros((128, K * 192), np.float32)
    for k in range(K):
        wbs0[:, k * 192:(k + 1) * 192] = bf[k][:, 0:192]
        wbs1[:, k * 192:(k + 1) * 192] = bf[k][:, 192:384]
    wbs0[0:64, K * 192:K * 192 + 40] = cw2.reshape(C, 40)
    wbs0[0:64, K * 192 + 40] = np.asarray(conv_b, np.float32)
    wbs0 = np.ascontiguousarray(wbs0).astype(ml_dtypes.bfloat16)
    wbs1 = np.ascontiguousarray(wbs1).astype(ml_dtypes.bfloat16)
    x = np.asarray(x, np.float32)
    xp = np.zeros((N, C, H, WP), np.float32)
    xp[:, :, :, :W] = x
    xs = xp.astype(ml_dtypes.bfloat16)
    in_maps = []
    for n in range(N):
        in_maps.append({
            "xin": np.ascontiguousarray(xs[n].reshape(C, HWELEM)),
            "wbs0": wbs0,
            "wbs1": wbs1,
        })
    return in_maps


def _run(inputs, trace=False, **kw):
    from concourse.bass_utils import run_bass_kernel_spmd
    nc = _get_nc()
    in_maps = _prep_inputs(**inputs)
    return run_bass_kernel_spmd(nc, in_maps, core_ids=list(range(N)), trace=trace, **kw)


def kernel(**inputs):
    res = _run(inputs)
    out = np.stack([np.asarray(res.results[n]["out"]) for n in range(N)]).astype(np.float32)
    return out


# revision 24
# speedup vs baseline: 1.2728x; 1.0601x over previous
"""CondConv2d on 8 Trainium2 NeuronCores — data-parallel over batch N=8.

v3 design (per core, one sample):
  - bf16 end-to-end on the wire; static residual conv folded into the
    expert banks host-side (W'_k = W_k + conv_w), softmax handled by
    normalizing exp(logits) before the weight mix.
  - Dual-plane conv: the PE's full 128 output columns = 64 channels x 2
    output-row planes.  Window (a, c) streams x rows [a, a+2] at column
    offset c; plane A (cols 0-63) computes out rows a+q from taps
    (0,c) [lower ci] + (+1,c) [row-shifted upper ci]; plane B (cols
    64-127) computes out rows a+q+1 from tap (-1,c) [lower ci].  All 9
    taps in 3 matmuls per 3-row tile (vs 6 for the single-plane form).
  - Eviction adds the two planes (plane B of tile j-1 supplies row 3j)
    plus conv bias, strips the 130-wide padding, writes packed bf16
    staging; one out-DMA per 4 tiles.
  - x lower copy is split across both hwdge queues and lands first
    (gates the attention sums); weight banks follow on sync; the
    row-shifted upper copy streams last in 16-row chunks, staying ahead
    of the conv's consumption.
"""
import os
import numpy as np

N, C, H, W = 8, 64, 128, 128
K = 4
WP = W + 2                 # padded row width (130)
NELEM = WP * WP + 2        # per-partition x buffer length (16902)
RPT = 3                    # output rows per PSUM tile
HWELEM = H * WP            # 16640
F3 = WP * RPT              # stream/psum free size (390)

# lower-copy chunks (elements per partition): sync gets 72 rows,
# scalar 56 (its queue starts later); the last chunk is small so the
# attention tail is short
L_SYNC = [(0, 4680), (4680, 4680)]                # rows 0-71
L_SCAL = [(9360, 5720), (15080, 1560)]            # rows 72-115, 116-127
# upper-copy chunks (16 rows each), alternated so the earliest-needed
# rows land first on the less-loaded queue
U_SCAL = [(0, 2080), (4160, 2080), (8320, 2080), (12480, 2080)]
U_SYNC = [(2080, 2080), (6240, 2080), (10400, 2080), (14560, 2080)]

TILES_PER_DMA = 4


# ----------------------------------------------------------------------------
# host-side prep
# ----------------------------------------------------------------------------
def _make_cw2(net0_w, net0_b, net1_w, net1_b, net2_w, net2_b):
    """CW2[c, b, k]: logits[k] = sum_{c,b} CW2[c,b,k] * basis[c,b].
    basis: 0=total, 1=row0, 2=row127, 3=col0, 4=col127,
           5..8=corners (00,0W,H0,HW), 9=const 1."""
    cw = np.zeros((C, 10, K), np.float64)
    scale = 1.0 / (C * H * W)
    for w_net, pads in ((net0_w, (0, 0, 0)), (net1_w, (1, 1, 1)), (net2_w, (2, 1, 1))):
        Kk, _, kd, kh, kw = w_net.shape
        pd, ph, pw = pads
        for i in range(kd):
            clo, chi = max(0, i - pd), min(C - 1, C - 1 + i - pd)
            cmask = np.zeros(C)
            cmask[clo:chi + 1] = 1.0
            for j in range(kh):
                hlo, hhi = max(0, j - ph), min(H - 1, H - 1 + j - ph)
                dropA = 0 if hlo == 1 else (127 if hhi == H - 2 else None)
                for l in range(kw):
                    wlo, whi = max(0, l - pw), min(W - 1, W - 1 + l - pw)
                    dropB = 0 if wlo == 1 else (127 if whi == W - 2 else None)
                    v = np.zeros(10)
                    v[0] = 1.0
                    if dropA == 0: v[1] = -1.0
                    if dropA == 127: v[2] = -1.0
                    if dropB == 0: v[3] = -1.0
                    if dropB == 127: v[4] = -1.0
                    if dropA is not None and dropB is not None:
                        v[{(0, 0): 5, (0, 127): 6, (127, 0): 7, (127, 127): 8}[(dropA, dropB)]] = 1.0
                    for k in range(Kk):
                        cw[:, :, k] += w_net[k, 0, i, j, l] * scale * np.outer(cmask, v)
    btot = (net0_b + net1_b + net2_b).astype(np.float64)
    cw[:, 9, :] += btot[None, :] / C
    return np.ascontiguousarray(cw.astype(np.float32))


def _make_bank3(Wt):
    """Wt (co, ci, 3, 3) -> (128, 3, 128) dual-plane stationary layout.
    Window w (col offset c=w-1): rows 0-63 = lower ci, rows 64-127 =
    row-shifted upper ci; cols 0-63 = plane A (out row a+q), cols
    64-127 = plane B (out row a+q+1)."""
    bank = np.zeros((128, 3, 128), np.float32)
    for w in range(3):
        bank[:64, w, 0:64] = Wt[:, :, 1, w].T     # A: tap (0, c)
        bank[64:, w, 0:64] = Wt[:, :, 2, w].T     # A: tap (+1, c)
        bank[:64, w, 64:128] = Wt[:, :, 0, w].T   # B: tap (-1, c)
    return bank


# ----------------------------------------------------------------------------
# device program
# ----------------------------------------------------------------------------
_NC_CACHE = {}


def _build_nc():
    import concourse.bacc as bacc
    import concourse.tile as tile
    from concourse import mybir

    f32 = mybir.dt.float32
    bf16 = mybir.dt.bfloat16
    Alu = mybir.AluOpType
    Ax = mybir.AxisListType
    Act = mybir.ActivationFunctionType

    nc = bacc.Bacc("TRN2", target_bir_lowering=False, debug=False,
                   enable_asserts=False, num_devices=N)
    xin = nc.dram_tensor("xin", [C, HWELEM], bf16, kind="ExternalInput")
    # weight banks [128, K, 384] + smalls (cw2 40 + convb 1 on partitions
    # 0-63 in the last 48 cols); loaded via gpsimd's software DGE so it
    # doesn't compete with the x stream on the hwdge queues
    wbs = nc.dram_tensor("wbs", [128, K * 384 + 48], bf16, kind="ExternalInput")
    outT = nc.dram_tensor("out", [C, H, W], bf16, kind="ExternalOutput")

    with tile.TileContext(nc) as tc:
        with tc.tile_pool(name="singles", bufs=1) as S, \
             tc.tile_pool(name="stage", bufs=2) as STG, \
             tc.tile_pool(name="cpsum", bufs=4, space="PSUM") as PS, \
             tc.tile_pool(name="spsum", bufs=1, space="PSUM") as PS1:

            XL = S.tile([128, NELEM], bf16)
            wb_sb = S.tile([128, K * 384 + 48], bf16)
            zrow = S.tile([128, 128], bf16)       # zeros, warmup lhs
            onesall = S.tile([C, 128], f32)       # ones, logits broadcast
            att_sb = S.tile([128, K], f32)        # exp(logits)
            attn = S.tile([128, K], f32)          # normalized attention
            M10 = S.tile([C, 10], f32)
            PARTS = S.tile([C, 8], f32)
            G = S.tile([C, K], f32)
            convb = S.tile([C, 1], f32)
            mw = S.tile([128, 3, 128], f32)
            mwb = S.tile([128, 3, 128], bf16)
            bplane = S.tile([64, H + 2, W], f32)  # B-plane staging by out row
            scr_d = S.tile([C, 2600], bf16)       # DVE fold scratch
            scr_a = S.tile([C, 2600], bf16)       # ACT span scratch
            scr_g = S.tile([C, 16], f32)          # G contraction scratch
            scr_c = S.tile([C, 128], f32)         # col-sum scratch
            rs_sum = S.tile([128, 1], f32)
            rs_inv = S.tile([128, 1], f32)

            wpsum = PS1.tile([128, 512], f32)
            psum_b = PS1.tile([128, K], f32)

            cw2v = wb_sb[0:64, K * 384:K * 384 + 40].rearrange(
                "p (b k) -> p b k", k=K)

            # --- constants / border zeroing ---
            nc.vector.memset(zrow, 0.0)
            nc.vector.memset(onesall, 1.0)
            nc.vector.memset(M10[:, 9:10], 1.0)
            nc.vector.memset(XL[0:64, 0:132], 0.0)
            nc.vector.memset(XL[0:64, 132 + HWELEM:NELEM], 0.0)
            nc.vector.memset(XL[64:128, 0:2], 0.0)
            nc.vector.memset(XL[64:128, 2 + HWELEM:NELEM], 0.0)

            # --- input DMAs (queue order == issue order == emission order) ---
            nc.gpsimd.dma_start(out=wb_sb, in_=wbs[:, :])
            for a, ln in L_SYNC:
                nc.sync.dma_start(out=XL[0:64, 132 + a:132 + a + ln],
                                  in_=xin[:, a:a + ln])
            for a, ln in L_SCAL:
                nc.scalar.dma_start(out=XL[0:64, 132 + a:132 + a + ln],
                                    in_=xin[:, a:a + ln])
            for (asc, lsc), (asy, lsy) in zip(U_SCAL, U_SYNC):
                nc.scalar.dma_start(out=XL[64:128, 2 + asc:2 + asc + lsc],
                                    in_=xin[:, asc:asc + lsc])
                nc.sync.dma_start(out=XL[64:128, 2 + asy:2 + asy + lsy],
                                  in_=xin[:, asy:asy + lsy])

            # --- PE warm-up (results discarded; zrow is all-zero) ---
            for i in range(8):
                nc.tensor.matmul(wpsum[:, 0:128], zrow, zrow, start=True, stop=True)

            # --- attention basis sums, per lower chunk ---
            lchunks = L_SYNC + L_SCAL
            for c, (a0, ln) in enumerate(lchunks):
                a = 132 + a0
                dl = (ln * 5 // 8) & ~1          # DVE share (even)
                h = dl // 2
                nc.vector.scalar_tensor_tensor(
                    out=scr_d[:, :h], in0=XL[0:64, a:a + h], scalar=1.0,
                    in1=XL[0:64, a + h:a + dl], op0=Alu.mult, op1=Alu.add,
                    accum_out=PARTS[:, c:c + 1])
                nc.scalar.activation(
                    out=scr_a[:, :ln - dl], in_=XL[0:64, a + dl:a + ln],
                    func=Act.Identity, bias=0.0, scale=1.0,
                    accum_out=PARTS[:, 4 + c:5 + c])
                nc.tensor.matmul(wpsum[:, 0:512], zrow[0:64, :],
                                 scr_d[0:64, 0:512], start=True, stop=True)

            # --- edge/corner sums (lower copy): rows+corners on DVE, cols on ACT ---
            nc.vector.tensor_reduce(out=M10[:, 1:2], in_=XL[0:64, 132:132 + W],
                                    axis=Ax.X, op=Alu.add)
            nc.vector.tensor_reduce(out=M10[:, 2:3], in_=XL[0:64, 16642:16642 + W],
                                    axis=Ax.X, op=Alu.add)
            col0 = XL[0:64, 132:132 + HWELEM].rearrange("p (r w) -> p r w", w=WP)[:, :, 0:1]
            nc.scalar.activation(out=scr_c[:, 0:128].rearrange("p (a b) -> p a b", b=1),
                                 in_=col0, func=Act.Identity, bias=0.0, scale=1.0,
                                 accum_out=M10[:, 3:4])
            col1 = XL[0:64, 259:259 + HWELEM].rearrange("p (r w) -> p r w", w=WP)[:, :, 0:1]
            nc.scalar.activation(out=scr_c[:, 0:128].rearrange("p (a b) -> p a b", b=1),
                                 in_=col1, func=Act.Identity, bias=0.0, scale=1.0,
                                 accum_out=M10[:, 4:5])
            nc.vector.tensor_copy(
                out=M10[:, 5:7].rearrange("p (a b) -> p a b", b=1),
                in_=XL[0:64, 132:132 + 254].rearrange("p (a b) -> p a b", b=127)[:, :, 0:1])
            nc.vector.tensor_copy(
                out=M10[:, 7:9].rearrange("p (a b) -> p a b", b=1),
                in_=XL[0:64, 16642:16642 + 254].rearrange("p (a b) -> p a b", b=127)[:, :, 0:1])

            # copy conv bias (bf16, embedded in wbs) to fp32
            nc.vector.tensor_copy(out=convb,
                                  in_=wb_sb[0:64, K * 384 + 40:K * 384 + 41])

            # fold span partials into basis column 0
            nc.vector.tensor_reduce(out=M10[:, 0:1], in_=PARTS, axis=Ax.X, op=Alu.add)

            # per-channel coefficient contraction: G[c,k] = sum_b M10[c,b]*CW2[c,b,k]
            for k in range(K):
                nc.vector.scalar_tensor_tensor(
                    out=scr_g[:, 0:10], in0=M10[:, :], scalar=1.0,
                    in1=cw2v[:, :, k], op0=Alu.mult, op1=Alu.mult,
                    accum_out=G[:, k:k + 1])

            # logits broadcast; exp + sum in one ACT op; normalize on DVE
            nc.tensor.matmul(psum_b, onesall, G, start=True, stop=True)
            nc.scalar.activation(out=att_sb, in_=psum_b, func=Act.Exp,
                                 accum_out=rs_sum)
            nc.vector.reciprocal(out=rs_inv, in_=rs_sum)
            nc.vector.tensor_scalar_mul(out=attn, in0=att_sb, scalar1=rs_inv)

            # --- weight mixing: mw = sum_k attn_k * bank'_k, one group per
            # conv window so the first matmul unblocks after group 0 ---
            wv = wb_sb[:, 0:K * 384].rearrange("p (k c) -> p k c", k=K)
            mwf = mw.rearrange("p a b -> p (a b)")
            mbf = mwb.rearrange("p a b -> p (a b)")
            for g in range(3):
                sl = slice(g * 128, g * 128 + 128)
                nc.vector.tensor_scalar_mul(
                    out=mwf[:, sl], in0=wv[:, 0, sl], scalar1=attn[:, 0:1])
                for k in range(1, K):
                    tgt = mbf if k == K - 1 else mwf
                    nc.vector.scalar_tensor_tensor(
                        out=tgt[:, sl], in0=wv[:, k, sl],
                        scalar=attn[:, k:k + 1], in1=mwf[:, sl],
                        op0=Alu.mult, op1=Alu.add)

            # --- main conv: 43 dual-plane PSUM tiles x 3 matmuls ---
            # Eviction: ACT copies plane B (PSUM parts 64-127) into bplane_j
            # aligned to output rows (bplane_j[q] = B-contribution of out row
            # 3j+q, i.e. B_j rows 0,1 -> bplane_j[1:3], B_j row 2 ->
            # bplane_{j+1}[0]); DVE then does ONE STT per tile:
            # stg = (A-plane + convb) + bplane.
            ntiles = (H + RPT - 1) // RPT
            stg = None
            nc.vector.memset(bplane[:, 0:1, :], 0.0)
            for j in range(ntiles):
                a = RPT * j
                pt = PS.tile([128, F3], f32, tag="cps", name=f"cps{j}")
                for w in range(3):
                    o = 131 + WP * a + (w - 1)
                    nc.tensor.matmul(pt, mwb[:, w, :], XL[:, o:o + F3],
                                     start=(w == 0), stop=(w == 2))
                nrows = min(RPT, H - a)
                gi = j % TILES_PER_DMA
                if gi == 0:
                    stg = STG.tile([64, TILES_PER_DMA * RPT * W], bf16,
                                   tag="stg", name=f"stg{j // TILES_PER_DMA}")
                dst = stg[:, gi * RPT * W: gi * RPT * W + nrows * W].rearrange(
                    "p (r w) -> p r w", w=W)
                ptA = pt[0:64, :].rearrange("p (r w) -> p r w", w=WP)[:, :, 1:1 + W]
                ptB = pt[64:128, :].rearrange("p (r w) -> p r w", w=WP)[:, :, 1:1 + W]
                # ACT: B_j rows 0..2 -> bplane rows 3j+1..3j+3 (one op)
                nc.scalar.activation(out=bplane[:, RPT * j + 1:RPT * j + 1 + nrows, :],
                                     in_=ptB[:, 0:nrows], func=Act.Identity,
                                     bias=0.0, scale=1.0)
                # DVE: stg = (A + convb) + bplane[3j:3j+nrows]
                nc.vector.scalar_tensor_tensor(
                    out=dst, in0=ptA[:, 0:nrows], scalar=convb,
                    in1=bplane[:, RPT * j:RPT * j + nrows, :],
                    op0=Alu.add, op1=Alu.add)
                if gi == TILES_PER_DMA - 1 or j == ntiles - 1:
                    g0row = (j - gi) * RPT
                    grows = min(H - g0row, (gi + 1) * RPT)
                    src = stg[:, :grows * W].rearrange("p (r w) -> p r w", w=W)
                    nc.sync.dma_start(out=outT[:, g0row:g0row + grows, :], in_=src)

    nc.compile()
    return nc


def _get_nc():
    if "nc" not in _NC_CACHE:
        _NC_CACHE["nc"] = _build_nc()
    return _NC_CACHE["nc"]


def _prep_inputs(x, weight, conv_w, conv_b, net0_w, net0_b, net1_w, net1_b,
                 net2_w, net2_b):
    import ml_dtypes
    cw2 = _make_cw2(np.asarray(net0_w, np.float32), np.asarray(net0_b, np.float32),
                    np.asarray(net1_w, np.float32), np.asarray(net1_b, np.float32),
                    np.asarray(net2_w, np.float32), np.asarray(net2_b, np.float32))
    wsum = np.asarray(weight, np.float32) + np.asarray(conv_w, np.float32)[None]
    banks = np.stack([_make_bank3(wsum[k]) for k in range(K)])  # (K,128,3,128)
    bf = banks.reshape(K, 128, 384)
    wbs = np.zeros((128, K * 384 + 48), np.float32)
    for k in range(K):
        wbs[:, k * 384:(k + 1) * 384] = bf[k]
    wbs[0:64, K * 384:K * 384 + 40] = cw2.reshape(C, 40)
    wbs[0:64, K * 384 + 40] = np.asarray(conv_b, np.float32)
    wbs = np.ascontiguousarray(wbs).astype(ml_dtypes.bfloat16)
    x = np.asarray(x, np.float32)
    xp = np.zeros((N, C, H, WP), np.float32)
    xp[:, :, :, :W] = x
    xs = xp.astype(ml_dtypes.bfloat16)
    in_maps = []
    for n in range(N):
        in_maps.append({
            "xin": np.ascontiguousarray(xs[n].reshape(C, HWELEM)),
            "wbs": wbs,
        })
    return in_maps


def _run(inputs, trace=False, **kw):
    from concourse.bass_utils import run_bass_kernel_spmd
    nc = _get_nc()
    in_maps = _prep_inputs(**inputs)
    return run_bass_kernel_spmd(nc, in_maps, core_ids=list(range(N)), trace=trace, **kw)


def kernel(**inputs):
    res = _run(inputs)
    out = np.stack([np.asarray(res.results[n]["out"]) for n in range(N)]).astype(np.float32)
    return out


# revision 29
# speedup vs baseline: 1.3063x; 1.0263x over previous
"""CondConv2d on 8 Trainium2 NeuronCores — data-parallel over batch N=8.

v3 design (per core, one sample):
  - bf16 end-to-end on the wire; static residual conv folded into the
    expert banks host-side (W'_k = W_k + conv_w), softmax handled by
    normalizing exp(logits) before the weight mix.
  - Dual-plane conv: the PE's full 128 output columns = 64 channels x 2
    output-row planes.  Window (a, c) streams x rows [a, a+2] at column
    offset c; plane A (cols 0-63) computes out rows a+q from taps
    (0,c) [lower ci] + (+1,c) [row-shifted upper ci]; plane B (cols
    64-127) computes out rows a+q+1 from tap (-1,c) [lower ci].  All 9
    taps in 3 matmuls per 3-row tile (vs 6 for the single-plane form).
  - Eviction adds the two planes (plane B of tile j-1 supplies row 3j)
    plus conv bias, strips the 130-wide padding, writes packed bf16
    staging; one out-DMA per 4 tiles.
  - x lower copy is split across both hwdge queues and lands first
    (gates the attention sums); weight banks follow on sync; the
    row-shifted upper copy streams last in 16-row chunks, staying ahead
    of the conv's consumption.
"""
import os
import numpy as np

N, C, H, W = 8, 64, 128, 128
K = 4
WP = W + 2                 # padded row width (130)
NELEM = WP * WP + 2        # per-partition x buffer length (16902)
RPT = 3                    # output rows per PSUM tile
HWELEM = H * WP            # 16640
F3 = WP * RPT              # stream/psum free size (390)

# lower-copy chunks (elements per partition): sync gets 72 rows,
# scalar 56 (its queue starts later)
L_SYNC = [(0, 4680), (4680, 4680)]                # rows 0-71
L_SCAL = [(9360, 3640), (13000, 3640)]            # rows 72-99, 100-127
# upper-copy chunks (16 rows each), alternated so the earliest-needed
# rows land first on the less-loaded queue
U_SCAL = [(0, 2080), (4160, 2080), (8320, 2080), (12480, 2080)]
U_SYNC = [(2080, 2080), (6240, 2080), (10400, 2080), (14560, 2080)]

TILES_PER_DMA = 6


# ----------------------------------------------------------------------------
# host-side prep
# ----------------------------------------------------------------------------
def _make_cw2(net0_w, net0_b, net1_w, net1_b, net2_w, net2_b):
    """CW2[c, b, k]: logits[k] = sum_{c,b} CW2[c,b,k] * basis[c,b].
    basis: 0=total, 1=row0, 2=row127, 3=col0, 4=col127,
           5..8=corners (00,0W,H0,HW), 9=const 1."""
    cw = np.zeros((C, 10, K), np.float64)
    scale = 1.0 / (C * H * W)
    for w_net, pads in ((net0_w, (0, 0, 0)), (net1_w, (1, 1, 1)), (net2_w, (2, 1, 1))):
        Kk, _, kd, kh, kw = w_net.shape
        pd, ph, pw = pads
        for i in range(kd):
            clo, chi = max(0, i - pd), min(C - 1, C - 1 + i - pd)
            cmask = np.zeros(C)
            cmask[clo:chi + 1] = 1.0
            for j in range(kh):
                hlo, hhi = max(0, j - ph), min(H - 1, H - 1 + j - ph)
                dropA = 0 if hlo == 1 else (127 if hhi == H - 2 else None)
                for l in range(kw):
                    wlo, whi = max(0, l - pw), min(W - 1, W - 1 + l - pw)
                    dropB = 0 if wlo == 1 else (127 if whi == W - 2 else None)
                    v = np.zeros(10)
                    v[0] = 1.0
                    if dropA == 0: v[1] = -1.0
                    if dropA == 127: v[2] = -1.0
                    if dropB == 0: v[3] = -1.0
                    if dropB == 127: v[4] = -1.0
                    if dropA is not None and dropB is not None:
                        v[{(0, 0): 5, (0, 127): 6, (127, 0): 7, (127, 127): 8}[(dropA, dropB)]] = 1.0
                    for k in range(Kk):
                        cw[:, :, k] += w_net[k, 0, i, j, l] * scale * np.outer(cmask, v)
    btot = (net0_b + net1_b + net2_b).astype(np.float64)
    cw[:, 9, :] += btot[None, :] / C
    return np.ascontiguousarray(cw.astype(np.float32))


def _make_bank3(Wt):
    """Wt (co, ci, 3, 3) -> (128, 3, 128) dual-plane stationary layout.
    Window w (col offset c=w-1): rows 0-63 = lower ci, rows 64-127 =
    row-shifted upper ci; cols 0-63 = plane A (out row a+q), cols
    64-127 = plane B (out row a+q+1)."""
    bank = np.zeros((128, 3, 128), np.float32)
    for w in range(3):
        bank[:64, w, 0:64] = Wt[:, :, 1, w].T     # A: tap (0, c)
        bank[64:, w, 0:64] = Wt[:, :, 2, w].T     # A: tap (+1, c)
        bank[:64, w, 64:128] = Wt[:, :, 0, w].T   # B: tap (-1, c)
    return bank


# ----------------------------------------------------------------------------
# device program
# ----------------------------------------------------------------------------
_NC_CACHE = {}


def _build_nc():
    import concourse.bacc as bacc
    import concourse.tile as tile
    from concourse import mybir

    f32 = mybir.dt.float32
    bf16 = mybir.dt.bfloat16
    Alu = mybir.AluOpType
    Ax = mybir.AxisListType
    Act = mybir.ActivationFunctionType

    nc = bacc.Bacc("TRN2", target_bir_lowering=False, debug=False,
                   enable_asserts=False, num_devices=N)
    xin = nc.dram_tensor("xin", [C, HWELEM], bf16, kind="ExternalInput")
    # weight banks [128, K, 384] + smalls (cw2 40 + convb 1 on partitions
    # 0-63 in the last 48 cols); loaded via gpsimd's software DGE so it
    # doesn't compete with the x stream on the hwdge queues
    wbs = nc.dram_tensor("wbs", [128, K * 384 + 48], bf16, kind="ExternalInput")
    outT = nc.dram_tensor("out", [C, H, W], bf16, kind="ExternalOutput")

    with tile.TileContext(nc) as tc:
        with tc.tile_pool(name="singles", bufs=1) as S, \
             tc.tile_pool(name="stage", bufs=3) as STG, \
             tc.tile_pool(name="cpsum", bufs=4, space="PSUM") as PS, \
             tc.tile_pool(name="spsum", bufs=1, space="PSUM") as PS1:

            XL = S.tile([128, NELEM], bf16)
            wb_sb = S.tile([128, K * 384 + 48], bf16)
            zrow = S.tile([128, 128], bf16)       # zeros, warmup lhs
            onesall = S.tile([C, 128], f32)       # ones, logits broadcast
            att_sb = S.tile([128, K], f32)        # exp(logits)
            attn = S.tile([128, K], f32)          # normalized attention
            M10 = S.tile([C, 10], f32)
            PARTS = S.tile([C, 8], f32)
            G = S.tile([C, K], f32)
            convb = S.tile([C, 1], f32)
            mw = S.tile([128, 3, 128], f32)
            mwb = S.tile([128, 3, 128], bf16)
            bplane = S.tile([64, H + 2, W], f32)  # B-plane staging by out row
            scr_d = S.tile([C, 2600], bf16)       # DVE fold scratch
            scr_a = S.tile([C, 2600], bf16)       # ACT span scratch
            scr_g = S.tile([C, 16], f32)          # G contraction scratch
            rs_sum = S.tile([128, 1], f32)
            rs_inv = S.tile([128, 1], f32)

            wpsum = PS1.tile([128, 512], f32)
            psum_b = PS1.tile([128, K], f32)

            cw2v = wb_sb[0:64, K * 384:K * 384 + 40].rearrange(
                "p (b k) -> p b k", k=K)

            # --- constants / border zeroing ---
            nc.vector.memset(zrow, 0.0)
            nc.vector.memset(onesall, 1.0)
            nc.vector.memset(M10[:, 9:10], 1.0)
            nc.vector.memset(XL[0:64, 0:132], 0.0)
            nc.vector.memset(XL[0:64, 132 + HWELEM:NELEM], 0.0)
            nc.vector.memset(XL[64:128, 0:2], 0.0)
            nc.vector.memset(XL[64:128, 2 + HWELEM:NELEM], 0.0)

            # --- input DMAs (queue order == issue order == emission order) ---
            nc.gpsimd.dma_start(out=wb_sb, in_=wbs[:, :])
            for a, ln in L_SYNC:
                nc.sync.dma_start(out=XL[0:64, 132 + a:132 + a + ln],
                                  in_=xin[:, a:a + ln])
            for a, ln in L_SCAL:
                nc.scalar.dma_start(out=XL[0:64, 132 + a:132 + a + ln],
                                    in_=xin[:, a:a + ln])
            for (asc, lsc), (asy, lsy) in zip(U_SCAL, U_SYNC):
                nc.scalar.dma_start(out=XL[64:128, 2 + asc:2 + asc + lsc],
                                    in_=xin[:, asc:asc + lsc])
                nc.sync.dma_start(out=XL[64:128, 2 + asy:2 + asy + lsy],
                                  in_=xin[:, asy:asy + lsy])

            # --- PE warm-up (results discarded; zrow is all-zero) ---
            for i in range(8):
                nc.tensor.matmul(wpsum[:, 0:128], zrow, zrow, start=True, stop=True)

            # --- attention basis sums, per lower chunk, in landing order ---
            lchunks = [L_SYNC[0], L_SCAL[0], L_SYNC[1], L_SCAL[1]]
            for c, (a0, ln) in enumerate(lchunks):
                a = 132 + a0
                dl = (ln * 5 // 8) & ~1          # DVE share (even)
                h = dl // 2
                nc.vector.scalar_tensor_tensor(
                    out=scr_d[:, :h], in0=XL[0:64, a:a + h], scalar=1.0,
                    in1=XL[0:64, a + h:a + dl], op0=Alu.mult, op1=Alu.add,
                    accum_out=PARTS[:, c:c + 1])
                nc.scalar.activation(
                    out=scr_a[:, :ln - dl], in_=XL[0:64, a + dl:a + ln],
                    func=Act.Identity, bias=0.0, scale=1.0,
                    accum_out=PARTS[:, 4 + c:5 + c])

            # keep the PE clock ramped while the load streams: dummy matmuls
            # gated directly on x-chunk DMA arrivals (no compute-engine deps)
            for a0, ln in (L_SYNC[0], L_SCAL[0]):
                nc.tensor.matmul(wpsum[:, 0:512], zrow[0:64, :],
                                 XL[0:64, 132 + a0:132 + a0 + 512],
                                 start=True, stop=True)
            for a0, ln in U_SCAL:
                nc.tensor.matmul(wpsum[:, 0:512], zrow[64:128, :],
                                 XL[64:128, 2 + a0:2 + a0 + 512],
                                 start=True, stop=True)

            # --- edge/corner sums (lower copy), all on DVE ---
            nc.vector.tensor_reduce(out=M10[:, 1:2], in_=XL[0:64, 132:132 + W],
                                    axis=Ax.X, op=Alu.add)
            nc.vector.tensor_reduce(out=M10[:, 2:3], in_=XL[0:64, 16642:16642 + W],
                                    axis=Ax.X, op=Alu.add)
            col0 = XL[0:64, 132:132 + HWELEM].rearrange("p (r w) -> p r w", w=WP)[:, :, 0:1]
            nc.vector.tensor_reduce(out=M10[:, 3:4], in_=col0, axis=Ax.XY, op=Alu.add)
            col1 = XL[0:64, 259:259 + HWELEM].rearrange("p (r w) -> p r w", w=WP)[:, :, 0:1]
            nc.vector.tensor_reduce(out=M10[:, 4:5], in_=col1, axis=Ax.XY, op=Alu.add)
            nc.vector.tensor_copy(
                out=M10[:, 5:7].rearrange("p (a b) -> p a b", b=1),
                in_=XL[0:64, 132:132 + 254].rearrange("p (a b) -> p a b", b=127)[:, :, 0:1])
            nc.vector.tensor_copy(
                out=M10[:, 7:9].rearrange("p (a b) -> p a b", b=1),
                in_=XL[0:64, 16642:16642 + 254].rearrange("p (a b) -> p a b", b=127)[:, :, 0:1])

            # copy conv bias (bf16, embedded in wbs) to fp32
            nc.vector.tensor_copy(out=convb,
                                  in_=wb_sb[0:64, K * 384 + 40:K * 384 + 41])

            # fold span partials into basis column 0
            nc.vector.tensor_reduce(out=M10[:, 0:1], in_=PARTS, axis=Ax.X, op=Alu.add)

            # per-channel coefficient contraction: G[c,k] = sum_b M10[c,b]*CW2[c,b,k]
            for k in range(K):
                nc.vector.scalar_tensor_tensor(
                    out=scr_g[:, 0:10], in0=M10[:, :], scalar=1.0,
                    in1=cw2v[:, :, k], op0=Alu.mult, op1=Alu.mult,
                    accum_out=G[:, k:k + 1])

            # logits broadcast; exp + sum in one ACT op; normalize on DVE
            nc.tensor.matmul(psum_b, onesall, G, start=True, stop=True)
            nc.scalar.activation(out=att_sb, in_=psum_b, func=Act.Exp,
                                 accum_out=rs_sum)
            nc.vector.reciprocal(out=rs_inv, in_=rs_sum)
            nc.vector.tensor_scalar_mul(out=attn, in0=att_sb, scalar1=rs_inv)

            # --- weight mixing: mw = sum_k attn_k * bank'_k, one group per
            # conv window so the first matmul unblocks after group 0 ---
            wv = wb_sb[:, 0:K * 384].rearrange("p (k c) -> p k c", k=K)
            mwf = mw.rearrange("p a b -> p (a b)")
            mbf = mwb.rearrange("p a b -> p (a b)")
            for g in range(3):
                sl = slice(g * 128, g * 128 + 128)
                nc.vector.tensor_scalar_mul(
                    out=mwf[:, sl], in0=wv[:, 0, sl], scalar1=attn[:, 0:1])
                for k in range(1, K):
                    tgt = mbf if k == K - 1 else mwf
                    nc.vector.scalar_tensor_tensor(
                        out=tgt[:, sl], in0=wv[:, k, sl],
                        scalar=attn[:, k:k + 1], in1=mwf[:, sl],
                        op0=Alu.mult, op1=Alu.add)

            # --- main conv: 43 dual-plane PSUM tiles x 3 matmuls ---
            # Eviction: ACT copies plane B (PSUM parts 64-127) into bplane_j
            # aligned to output rows (bplane_j[q] = B-contribution of out row
            # 3j+q, i.e. B_j rows 0,1 -> bplane_j[1:3], B_j row 2 ->
            # bplane_{j+1}[0]); DVE then does ONE STT per tile:
            # stg = (A-plane + convb) + bplane.
            ntiles = (H + RPT - 1) // RPT
            stg = None
            nc.vector.memset(bplane[:, 0:1, :], 0.0)
            for j in range(ntiles):
                a = RPT * j
                pt = PS.tile([128, F3], f32, tag="cps", name=f"cps{j}")
                for w in range(3):
                    o = 131 + WP * a + (w - 1)
                    nc.tensor.matmul(pt, mwb[:, w, :], XL[:, o:o + F3],
                                     start=(w == 0), stop=(w == 2))
                nrows = min(RPT, H - a)
                gi = j % TILES_PER_DMA
                if gi == 0:
                    stg = STG.tile([64, TILES_PER_DMA * RPT * W], bf16,
                                   tag="stg", name=f"stg{j // TILES_PER_DMA}")
                dst = stg[:, gi * RPT * W: gi * RPT * W + nrows * W].rearrange(
                    "p (r w) -> p r w", w=W)
                ptA = pt[0:64, :].rearrange("p (r w) -> p r w", w=WP)[:, :, 1:1 + W]
                ptB = pt[64:128, :].rearrange("p (r w) -> p r w", w=WP)[:, :, 1:1 + W]
                # ACT: B_j rows 0..2 -> bplane rows 3j+1..3j+3 (one op)
                nc.scalar.activation(out=bplane[:, RPT * j + 1:RPT * j + 1 + nrows, :],
                                     in_=ptB[:, 0:nrows], func=Act.Identity,
                                     bias=0.0, scale=1.0)
                # DVE: stg = (A + convb) + bplane[3j:3j+nrows]
                nc.vector.scalar_tensor_tensor(
                    out=dst, in0=ptA[:, 0:nrows], scalar=convb,
                    in1=bplane[:, RPT * j:RPT * j + nrows, :],
                    op0=Alu.add, op1=Alu.add)
                if gi == TILES_PER_DMA - 1 or j == ntiles - 1:
                    g0row = (j - gi) * RPT
                    grows = min(H - g0row, (gi + 1) * RPT)
                    src = stg[:, :grows * W].rearrange("p (r w) -> p r w", w=W)
                    eng = nc.sync if (j // TILES_PER_DMA) % 2 == 0 else nc.scalar
                    eng.dma_start(out=outT[:, g0row:g0row + grows, :], in_=src)

    nc.compile()
    return nc


def _get_nc():
    if "nc" not in _NC_CACHE:
        _NC_CACHE["nc"] = _build_nc()
    return _NC_CACHE["nc"]


def _prep_inputs(x, weight, conv_w, conv_b, net0_w, net0_b, net1_w, net1_b,
                 net2_w, net2_b):
    import ml_dtypes
    cw2 = _make_cw2(np.asarray(net0_w, np.float32), np.asarray(net0_b, np.float32),
                    np.asarray(net1_w, np.float32), np.asarray(net1_b, np.float32),
                    np.asarray(net2_w, np.float32), np.asarray(net2_b, np.float32))
    wsum = np.asarray(weight, np.float32) + np.asarray(conv_w, np.float32)[None]
    banks = np.stack([_make_bank3(wsum[k]) for k in range(K)])  # (K,128,3,128)
    bf = banks.reshape(K, 128, 384)
    wbs = np.zeros((128, K * 384 + 48), np.float32)
    for k in range(K):
        wbs[:, k * 384:(k + 1) * 384] = bf[k]
    wbs[0:64, K * 384:K * 384 + 40] = cw2.reshape(C, 40)
    wbs[0:64, K * 384 + 40] = np.asarray(conv_b, np.float32)
    wbs = np.ascontiguousarray(wbs).astype(ml_dtypes.bfloat16)
    x = np.asarray(x, np.float32)
    xp = np.zeros((N, C, H, WP), np.float32)
    xp[:, :, :, :W] = x
    xs = xp.astype(ml_dtypes.bfloat16)
    in_maps = []
    for n in range(N):
        in_maps.append({
            "xin": np.ascontiguousarray(xs[n].reshape(C, HWELEM)),
            "wbs": wbs,
        })
    return in_maps


def _run(inputs, trace=False, **kw):
    from concourse.bass_utils import run_bass_kernel_spmd
    nc = _get_nc()
    in_maps = _prep_inputs(**inputs)
    return run_bass_kernel_spmd(nc, in_maps, core_ids=list(range(N)), trace=trace, **kw)


def kernel(**inputs):
    res = _run(inputs)
    out = np.stack([np.asarray(res.results[n]["out"]) for n in range(N)]).astype(np.float32)
    return out


# revision 34
# speedup vs baseline: 1.3369x; 1.0234x over previous
"""CondConv2d on 8 Trainium2 NeuronCores — data-parallel over batch N=8.

v3 design (per core, one sample):
  - bf16 end-to-end on the wire; static residual conv folded into the
    expert banks host-side (W'_k = W_k + conv_w), softmax handled by
    normalizing exp(logits) before the weight mix.
  - Dual-plane conv: the PE's full 128 output columns = 64 channels x 2
    output-row planes.  Window (a, c) streams x rows [a, a+2] at column
    offset c; plane A (cols 0-63) computes out rows a+q from taps
    (0,c) [lower ci] + (+1,c) [row-shifted upper ci]; plane B (cols
    64-127) computes out rows a+q+1 from tap (-1,c) [lower ci].  All 9
    taps in 3 matmuls per 3-row tile (vs 6 for the single-plane form).
  - Eviction adds the two planes (plane B of tile j-1 supplies row 3j)
    plus conv bias, strips the 130-wide padding, writes packed bf16
    staging; one out-DMA per 4 tiles.
  - x lower copy is split across both hwdge queues and lands first
    (gates the attention sums); weight banks follow on sync; the
    row-shifted upper copy streams last in 16-row chunks, staying ahead
    of the conv's consumption.
"""
import os
import numpy as np

N, C, H, W = 8, 64, 128, 128
K = 4
WP = W + 2                 # padded row width (130)
NELEM = WP * WP + 2        # per-partition x buffer length (16902)
RPT = 3                    # output rows per PSUM tile
HWELEM = H * WP            # 16640
F3 = WP * RPT              # stream/psum free size (390)

# lower-copy chunks (elements per partition): sync gets 72 rows,
# scalar 56 (its queue starts later)
L_SYNC = [(0, 4680), (4680, 4680)]                # rows 0-71
L_SCAL = [(9360, 3640), (13000, 3640)]            # rows 72-99, 100-127
# upper-copy chunks (16 rows each), alternated so the earliest-needed
# rows land first on the less-loaded queue
U_SCAL = [(0, 2080), (4160, 2080), (8320, 2080), (12480, 2080)]
U_SYNC = [(2080, 2080), (6240, 2080), (10400, 2080), (14560, 2080)]

TILES_PER_DMA = 6


# ----------------------------------------------------------------------------
# host-side prep
# ----------------------------------------------------------------------------
def _make_cw2(net0_w, net0_b, net1_w, net1_b, net2_w, net2_b):
    """CW2[c, b, k]: logits[k] = sum_{c,b} CW2[c,b,k] * basis[c,b].
    basis: 0=total, 1=row0, 2=row127, 3=col0, 4=col127,
           5..8=corners (00,0W,H0,HW), 9=const 1."""
    cw = np.zeros((C, 10, K), np.float64)
    scale = 1.0 / (C * H * W)
    for w_net, pads in ((net0_w, (0, 0, 0)), (net1_w, (1, 1, 1)), (net2_w, (2, 1, 1))):
        Kk, _, kd, kh, kw = w_net.shape
        pd, ph, pw = pads
        for i in range(kd):
            clo, chi = max(0, i - pd), min(C - 1, C - 1 + i - pd)
            cmask = np.zeros(C)
            cmask[clo:chi + 1] = 1.0
            for j in range(kh):
                hlo, hhi = max(0, j - ph), min(H - 1, H - 1 + j - ph)
                dropA = 0 if hlo == 1 else (127 if hhi == H - 2 else None)
                for l in range(kw):
                    wlo, whi = max(0, l - pw), min(W - 1, W - 1 + l - pw)
                    dropB = 0 if wlo == 1 else (127 if whi == W - 2 else None)
                    v = np.zeros(10)
                    v[0] = 1.0
                    if dropA == 0: v[1] = -1.0
                    if dropA == 127: v[2] = -1.0
                    if dropB == 0: v[3] = -1.0
                    if dropB == 127: v[4] = -1.0
                    if dropA is not None and dropB is not None:
                        v[{(0, 0): 5, (0, 127): 6, (127, 0): 7, (127, 127): 8}[(dropA, dropB)]] = 1.0
                    for k in range(Kk):
                        cw[:, :, k] += w_net[k, 0, i, j, l] * scale * np.outer(cmask, v)
    btot = (net0_b + net1_b + net2_b).astype(np.float64)
    cw[:, 9, :] += btot[None, :] / C
    return np.ascontiguousarray(cw.astype(np.float32))


def _make_bank3(Wt):
    """Wt (co, ci, 3, 3) -> (128, 3, 128) dual-plane stationary layout.
    Window w (col offset c=w-1): rows 0-63 = lower ci, rows 64-127 =
    row-shifted upper ci; cols 0-63 = plane A (out row a+q), cols
    64-127 = plane B (out row a+q+1)."""
    bank = np.zeros((128, 3, 128), np.float32)
    for w in range(3):
        bank[:64, w, 0:64] = Wt[:, :, 1, w].T     # A: tap (0, c)
        bank[64:, w, 0:64] = Wt[:, :, 2, w].T     # A: tap (+1, c)
        bank[:64, w, 64:128] = Wt[:, :, 0, w].T   # B: tap (-1, c)
    return bank


# ----------------------------------------------------------------------------
# device program
# ----------------------------------------------------------------------------
_NC_CACHE = {}


def _build_nc():
    import concourse.bacc as bacc
    import concourse.tile as tile
    from concourse import mybir

    f32 = mybir.dt.float32
    bf16 = mybir.dt.bfloat16
    Alu = mybir.AluOpType
    Ax = mybir.AxisListType
    Act = mybir.ActivationFunctionType

    nc = bacc.Bacc("TRN2", target_bir_lowering=False, debug=False,
                   enable_asserts=False, num_devices=N)
    xin = nc.dram_tensor("xin", [C, HWELEM], bf16, kind="ExternalInput")
    # weight banks [128, K, 384] + smalls (cw2 40 + convb 1 on partitions
    # 0-63 in the last 48 cols); loaded via gpsimd's software DGE so it
    # doesn't compete with the x stream on the hwdge queues
    wbs = nc.dram_tensor("wbs", [128, K * 384 + 48], bf16, kind="ExternalInput")
    # output keeps the 130-wide padded rows (host strips cols [1:129]) so
    # the whole eviction path reads/writes contiguously
    outT = nc.dram_tensor("out", [C, H, WP], bf16, kind="ExternalOutput")

    with tile.TileContext(nc) as tc:
        with tc.tile_pool(name="singles", bufs=1) as S, \
             tc.tile_pool(name="stage", bufs=3) as STG, \
             tc.tile_pool(name="cpsum", bufs=4, space="PSUM") as PS, \
             tc.tile_pool(name="spsum", bufs=1, space="PSUM") as PS1:

            XL = S.tile([128, NELEM], bf16)
            wb_sb = S.tile([128, K * 384 + 48], bf16)
            zrow = S.tile([128, 128], bf16)       # zeros, warmup lhs
            onesall = S.tile([C, 128], f32)       # ones, logits broadcast
            att_sb = S.tile([128, K], f32)        # exp(logits)
            attn = S.tile([128, K], f32)          # normalized attention
            M10 = S.tile([C, 10], f32)
            PARTS = S.tile([C, 8], f32)
            G = S.tile([C, K], f32)
            convb = S.tile([C, 1], f32)
            mw = S.tile([128, 3, 128], f32)
            mwb = S.tile([128, 3, 128], bf16)
            bplane = S.tile([64, (H + 2) * WP], f32)  # B-plane staging by out row
            scr_d = S.tile([C, 2600], bf16)       # DVE fold scratch
            scr_a = S.tile([C, 2600], bf16)       # ACT span scratch
            scr_g = S.tile([C, 16], f32)          # G contraction scratch
            rs_sum = S.tile([128, 1], f32)
            rs_inv = S.tile([128, 1], f32)

            wpsum = PS1.tile([128, 512], f32)
            psum_b = PS1.tile([128, K], f32)

            cw2v = wb_sb[0:64, K * 384:K * 384 + 40].rearrange(
                "p (b k) -> p b k", k=K)

            # --- constants / border zeroing ---
            nc.vector.memset(zrow, 0.0)
            nc.vector.memset(onesall, 1.0)
            nc.vector.memset(M10[:, 9:10], 1.0)
            nc.vector.memset(XL[0:64, 0:132], 0.0)
            nc.vector.memset(XL[0:64, 132 + HWELEM:NELEM], 0.0)
            nc.vector.memset(XL[64:128, 0:2], 0.0)
            nc.vector.memset(XL[64:128, 2 + HWELEM:NELEM], 0.0)

            # --- input DMAs (queue order == issue order == emission order) ---
            nc.gpsimd.dma_start(out=wb_sb, in_=wbs[:, :])
            for a, ln in L_SYNC:
                nc.sync.dma_start(out=XL[0:64, 132 + a:132 + a + ln],
                                  in_=xin[:, a:a + ln])
            for a, ln in L_SCAL:
                nc.scalar.dma_start(out=XL[0:64, 132 + a:132 + a + ln],
                                    in_=xin[:, a:a + ln])
            for (asc, lsc), (asy, lsy) in zip(U_SCAL, U_SYNC):
                nc.scalar.dma_start(out=XL[64:128, 2 + asc:2 + asc + lsc],
                                    in_=xin[:, asc:asc + lsc])
                nc.sync.dma_start(out=XL[64:128, 2 + asy:2 + asy + lsy],
                                  in_=xin[:, asy:asy + lsy])

            # --- PE warm-up (results discarded; zrow is all-zero) ---
            for i in range(8):
                nc.tensor.matmul(wpsum[:, 0:128], zrow, zrow, start=True, stop=True)

            # --- attention basis sums, per lower chunk, in landing order ---
            lchunks = [L_SYNC[0], L_SCAL[0], L_SYNC[1], L_SCAL[1]]
            for c, (a0, ln) in enumerate(lchunks):
                a = 132 + a0
                dl = (ln * 5 // 8) & ~1          # DVE share (even)
                h = dl // 2
                nc.vector.scalar_tensor_tensor(
                    out=scr_d[:, :h], in0=XL[0:64, a:a + h], scalar=1.0,
                    in1=XL[0:64, a + h:a + dl], op0=Alu.mult, op1=Alu.add,
                    accum_out=PARTS[:, c:c + 1])
                nc.scalar.activation(
                    out=scr_a[:, :ln - dl], in_=XL[0:64, a + dl:a + ln],
                    func=Act.Identity, bias=0.0, scale=1.0,
                    accum_out=PARTS[:, 4 + c:5 + c])

            # keep the PE clock ramped while the load streams: dummy matmuls
            # gated directly on early x-chunk DMA arrivals
            for a0, ln in (L_SYNC[0], L_SCAL[0], L_SYNC[1], L_SCAL[1]):
                nc.tensor.matmul(wpsum[:, 0:512], zrow[0:64, :],
                                 XL[0:64, 132 + a0:132 + a0 + 512],
                                 start=True, stop=True)

            # --- edge/corner sums (lower copy), all on DVE ---
            nc.vector.tensor_reduce(out=M10[:, 1:2], in_=XL[0:64, 132:132 + W],
                                    axis=Ax.X, op=Alu.add)
            nc.vector.tensor_reduce(out=M10[:, 2:3], in_=XL[0:64, 16642:16642 + W],
                                    axis=Ax.X, op=Alu.add)
            col0 = XL[0:64, 132:132 + HWELEM].rearrange("p (r w) -> p r w", w=WP)[:, :, 0:1]
            nc.vector.tensor_reduce(out=M10[:, 3:4], in_=col0, axis=Ax.XY, op=Alu.add)
            col1 = XL[0:64, 259:259 + HWELEM].rearrange("p (r w) -> p r w", w=WP)[:, :, 0:1]
            nc.vector.tensor_reduce(out=M10[:, 4:5], in_=col1, axis=Ax.XY, op=Alu.add)
            nc.vector.tensor_copy(
                out=M10[:, 5:7].rearrange("p (a b) -> p a b", b=1),
                in_=XL[0:64, 132:132 + 254].rearrange("p (a b) -> p a b", b=127)[:, :, 0:1])
            nc.vector.tensor_copy(
                out=M10[:, 7:9].rearrange("p (a b) -> p a b", b=1),
                in_=XL[0:64, 16642:16642 + 254].rearrange("p (a b) -> p a b", b=127)[:, :, 0:1])

            # copy conv bias (bf16, embedded in wbs) to fp32
            nc.vector.tensor_copy(out=convb,
                                  in_=wb_sb[0:64, K * 384 + 40:K * 384 + 41])

            # fold span partials into basis column 0
            nc.vector.tensor_reduce(out=M10[:, 0:1], in_=PARTS, axis=Ax.X, op=Alu.add)

            # per-channel coefficient contraction: G[c,k] = sum_b M10[c,b]*CW2[c,b,k]
            for k in range(K):
                nc.vector.scalar_tensor_tensor(
                    out=scr_g[:, 0:10], in0=M10[:, :], scalar=1.0,
                    in1=cw2v[:, :, k], op0=Alu.mult, op1=Alu.mult,
                    accum_out=G[:, k:k + 1])

            # logits broadcast; exp + sum in one ACT op; normalize on DVE
            nc.tensor.matmul(psum_b, onesall, G, start=True, stop=True)
            nc.scalar.activation(out=att_sb, in_=psum_b, func=Act.Exp,
                                 accum_out=rs_sum)
            nc.vector.reciprocal(out=rs_inv, in_=rs_sum)
            nc.vector.tensor_scalar_mul(out=attn, in0=att_sb, scalar1=rs_inv)

            # --- weight mixing: mw = sum_k attn_k * bank'_k, one group per
            # conv window so the first matmul unblocks after group 0 ---
            wv = wb_sb[:, 0:K * 384].rearrange("p (k c) -> p k c", k=K)
            mwf = mw.rearrange("p a b -> p (a b)")
            mbf = mwb.rearrange("p a b -> p (a b)")
            for g in range(3):
                sl = slice(g * 128, g * 128 + 128)
                nc.vector.tensor_scalar_mul(
                    out=mwf[:, sl], in0=wv[:, 0, sl], scalar1=attn[:, 0:1])
                for k in range(1, K):
                    tgt = mbf if k == K - 1 else mwf
                    nc.vector.scalar_tensor_tensor(
                        out=tgt[:, sl], in0=wv[:, k, sl],
                        scalar=attn[:, k:k + 1], in1=mwf[:, sl],
                        op0=Alu.mult, op1=Alu.add)

            # --- main conv: 43 dual-plane PSUM tiles x 3 matmuls ---
            # Eviction: ACT copies plane B (PSUM parts 64-127) into bplane_j
            # aligned to output rows (bplane_j[q] = B-contribution of out row
            # 3j+q, i.e. B_j rows 0,1 -> bplane_j[1:3], B_j row 2 ->
            # bplane_{j+1}[0]); DVE then does ONE STT per tile:
            # stg = (A-plane + convb) + bplane.
            # tile -> out-DMA group map: groups of 6 tiles, smaller trailing
            # groups so the final DMA (and the kernel tail) is short
            grp_of = {}
            gstart = {}
            gg = 0
            jj = 0
            for size in (6, 6, 6, 6, 6, 6, 3, 3, 1):
                gstart[gg] = jj
                for _ in range(size):
                    grp_of[jj] = gg
                    jj += 1
                gg += 1
            ntiles = (H + RPT - 1) // RPT
            stg = None
            nc.vector.memset(bplane[:, 0:WP], 0.0)
            for j in range(ntiles):
                a = RPT * j
                pt = PS.tile([128, F3], f32, tag="cps", name=f"cps{j}")
                for w in range(3):
                    o = 131 + WP * a + (w - 1)
                    nc.tensor.matmul(pt, mwb[:, w, :], XL[:, o:o + F3],
                                     start=(w == 0), stop=(w == 2))
                nrows = min(RPT, H - a)
                g = grp_of[j]
                gi = j - gstart[g]
                if gi == 0:
                    stg = STG.tile([64, 6 * RPT * WP], bf16,
                                   tag="stg", name=f"stg{g}")
                dst = stg[:, gi * RPT * WP: gi * RPT * WP + nrows * WP]
                # ACT: B_j rows 0..2 -> bplane rows 3j+1..3j+3 (contiguous)
                nc.scalar.activation(
                    out=bplane[:, (RPT * j + 1) * WP:(RPT * j + 1 + nrows) * WP],
                    in_=pt[64:128, 0:nrows * WP], func=Act.Identity,
                    bias=0.0, scale=1.0)
                # DVE: stg = (A + convb) + bplane[3j:3j+nrows] (contiguous)
                nc.vector.scalar_tensor_tensor(
                    out=dst, in0=pt[0:64, 0:nrows * WP], scalar=convb,
                    in1=bplane[:, RPT * j * WP:(RPT * j + nrows) * WP],
                    op0=Alu.add, op1=Alu.add)
                if j == ntiles - 1 or grp_of[j + 1] != g:
                    g0row = gstart[g] * RPT
                    grows = min(H - g0row, (j + 1 - gstart[g]) * RPT)
                    src = stg[:, :grows * WP].rearrange("p (r w) -> p r w", w=WP)
                    eng = nc.sync if g % 2 == 0 else nc.scalar
                    eng.dma_start(out=outT[:, g0row:g0row + grows, :], in_=src)

    nc.compile()
    return nc


def _get_nc():
    if "nc" not in _NC_CACHE:
        _NC_CACHE["nc"] = _build_nc()
    return _NC_CACHE["nc"]


def _prep_inputs(x, weight, conv_w, conv_b, net0_w, net0_b, net1_w, net1_b,
                 net2_w, net2_b):
    import ml_dtypes
    cw2 = _make_cw2(np.asarray(net0_w, np.float32), np.asarray(net0_b, np.float32),
                    np.asarray(net1_w, np.float32), np.asarray(net1_b, np.float32),
                    np.asarray(net2_w, np.float32), np.asarray(net2_b, np.float32))
    wsum = np.asarray(weight, np.float32) + np.asarray(conv_w, np.float32)[None]
    banks = np.stack([_make_bank3(wsum[k]) for k in range(K)])  # (K,128,3,128)
    bf = banks.reshape(K, 128, 384)
    wbs = np.zeros((128, K * 384 + 48), np.float32)
    for k in range(K):
        wbs[:, k * 384:(k + 1) * 384] = bf[k]
    wbs[0:64, K * 384:K * 384 + 40] = cw2.reshape(C, 40)
    wbs[0:64, K * 384 + 40] = np.asarray(conv_b, np.float32)
    wbs = np.ascontiguousarray(wbs).astype(ml_dtypes.bfloat16)
    x = np.asarray(x, np.float32)
    xp = np.zeros((N, C, H, WP), np.float32)
    xp[:, :, :, :W] = x
    xs = xp.astype(ml_dtypes.bfloat16)
    in_maps = []
    for n in range(N):
        in_maps.append({
            "xin": np.ascontiguousarray(xs[n].reshape(C, HWELEM)),
            "wbs": wbs,
        })
    return in_maps


def _run(inputs, trace=False, **kw):
    from concourse.bass_utils import run_bass_kernel_spmd
    nc = _get_nc()
    in_maps = _prep_inputs(**inputs)
    return run_bass_kernel_spmd(nc, in_maps, core_ids=list(range(N)), trace=trace, **kw)


def kernel(**inputs):
    res = _run(inputs)
    out = np.stack([np.asarray(res.results[n]["out"]) for n in range(N)]).astype(np.float32)
    return np.ascontiguousarray(out[:, :, :, 1:1 + W])


# revision 41
# speedup vs baseline: 1.4436x; 1.0798x over previous
"""CondConv2d on 8 Trainium2 NeuronCores — data-parallel over batch N=8.

v3 design (per core, one sample):
  - bf16 end-to-end on the wire; static residual conv folded into the
    expert banks host-side (W'_k = W_k + conv_w), softmax handled by
    normalizing exp(logits) before the weight mix.
  - Dual-plane conv: the PE's full 128 output columns = 64 channels x 2
    output-row planes.  Window (a, c) streams x rows [a, a+2] at column
    offset c; plane A (cols 0-63) computes out rows a+q from taps
    (0,c) [lower ci] + (+1,c) [row-shifted upper ci]; plane B (cols
    64-127) computes out rows a+q+1 from tap (-1,c) [lower ci].  All 9
    taps in 3 matmuls per 3-row tile (vs 6 for the single-plane form).
  - Eviction adds the two planes (plane B of tile j-1 supplies row 3j)
    plus conv bias, strips the 130-wide padding, writes packed bf16
    staging; one out-DMA per 4 tiles.
  - x lower copy is split across both hwdge queues and lands first
    (gates the attention sums); weight banks follow on sync; the
    row-shifted upper copy streams last in 16-row chunks, staying ahead
    of the conv's consumption.
"""
import os
import numpy as np

N, C, H, W = 8, 64, 128, 128
K = 4
WP = W + 2                 # padded row width (130)
NELEM = WP * WP + 2        # per-partition x buffer length (16902)
RPT = 3                    # output rows per PSUM tile
HWELEM = H * WP            # 16640
F3 = WP * RPT              # stream/psum free size (390)

# lower-copy chunks (elements per partition).  Rows 0-8 and 96-128 land
# first so every edge/corner reduction depends only on early chunks (a
# late-dep op at the head of the in-order DVE stream would block all
# span folds behind it).  sync carries 76 rows, scalar 52 (its queue
# starts ~2.4us later).
L_SYNC = [(0, 1040), (12480, 4160), (1040, 4680)]   # r0-8, r96-128, r8-44
L_SCAL = [(5720, 4160), (9880, 2600)]               # r44-76, r76-96
# upper-copy 16-row chunks: sync issues most of them (its engine has no
# compute duties, so ring-full stalls on dma_start are harmless); scalar
# issues two late ones after its attention work
U_SYNC = [(0, 2080), (4160, 2080), (6240, 2080), (10400, 2080),
          (12480, 2080), (14560, 2080)]
U_SCAL = [(2080, 2080), (8320, 2080)]


# ----------------------------------------------------------------------------
# host-side prep
# ----------------------------------------------------------------------------
def _make_cw2(net0_w, net0_b, net1_w, net1_b, net2_w, net2_b):
    """CW2[c, b, k]: logits[k] = sum_{c,b} CW2[c,b,k] * basis[c,b].
    basis: 0=total, 1=row0, 2=row127, 3=col0, 4=col127,
           5..8=corners (00,0W,H0,HW), 9=const 1."""
    cw = np.zeros((C, 10, K), np.float64)
    scale = 1.0 / (C * H * W)
    for w_net, pads in ((net0_w, (0, 0, 0)), (net1_w, (1, 1, 1)), (net2_w, (2, 1, 1))):
        Kk, _, kd, kh, kw = w_net.shape
        pd, ph, pw = pads
        for i in range(kd):
            clo, chi = max(0, i - pd), min(C - 1, C - 1 + i - pd)
            cmask = np.zeros(C)
            cmask[clo:chi + 1] = 1.0
            for j in range(kh):
                hlo, hhi = max(0, j - ph), min(H - 1, H - 1 + j - ph)
                dropA = 0 if hlo == 1 else (127 if hhi == H - 2 else None)
                for l in range(kw):
                    wlo, whi = max(0, l - pw), min(W - 1, W - 1 + l - pw)
                    dropB = 0 if wlo == 1 else (127 if whi == W - 2 else None)
                    v = np.zeros(10)
                    v[0] = 1.0
                    if dropA == 0: v[1] = -1.0
                    if dropA == 127: v[2] = -1.0
                    if dropB == 0: v[3] = -1.0
                    if dropB == 127: v[4] = -1.0
                    if dropA is not None and dropB is not None:
                        v[{(0, 0): 5, (0, 127): 6, (127, 0): 7, (127, 127): 8}[(dropA, dropB)]] = 1.0
                    for k in range(Kk):
                        cw[:, :, k] += w_net[k, 0, i, j, l] * scale * np.outer(cmask, v)
    btot = (net0_b + net1_b + net2_b).astype(np.float64)
    cw[:, 9, :] += btot[None, :] / C
    return np.ascontiguousarray(cw.astype(np.float32))


def _make_bank3(Wt):
    """Wt (co, ci, 3, 3) -> (128, 3, 128) dual-plane stationary layout.
    Window w (col offset c=w-1): rows 0-63 = lower ci, rows 64-127 =
    row-shifted upper ci; cols 0-63 = plane A (out row a+q), cols
    64-127 = plane B (out row a+q+1)."""
    bank = np.zeros((128, 3, 128), np.float32)
    for w in range(3):
        bank[:64, w, 0:64] = Wt[:, :, 1, w].T     # A: tap (0, c)
        bank[64:, w, 0:64] = Wt[:, :, 2, w].T     # A: tap (+1, c)
        bank[:64, w, 64:128] = Wt[:, :, 0, w].T   # B: tap (-1, c)
    return bank


# ----------------------------------------------------------------------------
# device program
# ----------------------------------------------------------------------------
_NC_CACHE = {}


def _build_nc():
    import concourse.bacc as bacc
    import concourse.tile as tile
    from concourse import mybir

    f32 = mybir.dt.float32
    bf16 = mybir.dt.bfloat16
    Alu = mybir.AluOpType
    Ax = mybir.AxisListType
    Act = mybir.ActivationFunctionType

    nc = bacc.Bacc("TRN2", target_bir_lowering=False, debug=False,
                   enable_asserts=False, num_devices=N)
    xin = nc.dram_tensor("xin", [C, HWELEM], bf16, kind="ExternalInput")
    # weight banks [128, K, 384] + smalls (cw2 40 + convb 1 on partitions
    # 0-63 in the last 48 cols); loaded via gpsimd's software DGE so it
    # doesn't compete with the x stream on the hwdge queues
    wbs = nc.dram_tensor("wbs", [128, K * 384 + 48], bf16, kind="ExternalInput")
    # output keeps the 130-wide padded rows (host strips cols [1:129]) so
    # the whole eviction path reads/writes contiguously
    outT = nc.dram_tensor("out", [C, H, WP], bf16, kind="ExternalOutput")

    with tile.TileContext(nc) as tc:
        with tc.tile_pool(name="singles", bufs=1) as S, \
             tc.tile_pool(name="stage", bufs=3) as STG, \
             tc.tile_pool(name="cpsum", bufs=5, space="PSUM") as PS, \
             tc.tile_pool(name="spsum", bufs=1, space="PSUM") as PS1:

            XL = S.tile([128, NELEM], bf16)
            wb_sb = S.tile([128, K * 384 + 48], bf16)
            zrow = S.tile([128, 128], bf16)       # zeros, warmup lhs
            onesall = S.tile([C, 128], f32)       # ones, logits broadcast
            att_sb = S.tile([128, K], f32)        # exp(logits)
            attn = S.tile([128, K], f32)          # normalized attention
            M10 = S.tile([C, 10], f32)
            PARTS = S.tile([C, 10], f32)
            COLP = S.tile([C, 10], f32)
            G = S.tile([C, K], f32)
            convb = S.tile([C, 1], f32)
            mw = S.tile([128, 3, 128], f32)
            mwb = S.tile([128, 3, 128], bf16)
            bplane = S.tile([64, (H + 2) * WP], f32)  # B-plane staging by out row
            scr_d = S.tile([C, 2600], bf16)       # DVE fold scratch
            scr_a = S.tile([C, 2600], bf16)       # ACT span scratch
            scr_g = S.tile([C, 16], f32)          # G contraction scratch
            rs_sum = S.tile([128, 1], f32)
            rs_inv = S.tile([128, 1], f32)

            wpsum = PS1.tile([128, 512], f32)
            psum_b = PS1.tile([128, K], f32)

            cw2v = wb_sb[0:64, K * 384:K * 384 + 40].rearrange(
                "p (b k) -> p b k", k=K)

            # --- constants / border zeroing ---
            nc.vector.memset(zrow, 0.0)
            nc.vector.memset(onesall, 1.0)
            nc.vector.memset(M10[:, 9:10], 1.0)
            nc.vector.memset(XL[0:64, 0:132], 0.0)
            nc.vector.memset(XL[0:64, 132 + HWELEM:NELEM], 0.0)
            nc.vector.memset(XL[64:128, 0:2], 0.0)
            nc.vector.memset(XL[64:128, 2 + HWELEM:NELEM], 0.0)

            # --- input DMAs (queue order == issue order == emission order) ---
            nc.gpsimd.dma_start(out=wb_sb, in_=wbs[:, :])
            for a, ln in L_SYNC:
                nc.sync.dma_start(out=XL[0:64, 132 + a:132 + a + ln],
                                  in_=xin[:, a:a + ln])
            for a, ln in L_SCAL:
                nc.scalar.dma_start(out=XL[0:64, 132 + a:132 + a + ln],
                                    in_=xin[:, a:a + ln])
            for a, ln in U_SYNC:
                nc.sync.dma_start(out=XL[64:128, 2 + a:2 + a + ln],
                                  in_=xin[:, a:a + ln])
            # (U_SCAL issues are emitted after the ACT attention work below)

            # --- PE warm-up (results discarded; zrow is all-zero) ---
            for i in range(8):
                nc.tensor.matmul(wpsum[:, 0:128], zrow, zrow, start=True, stop=True)

            # --- attention basis sums, per lower chunk, in landing order.
            # Each chunk also contributes its own col0/col127 partials so no
            # strided reduce depends on a late chunk. ---
            lchunks = [L_SYNC[0], L_SYNC[1], L_SCAL[0], L_SYNC[2], L_SCAL[1]]
            for c, (a0, ln) in enumerate(lchunks):
                a = 132 + a0
                dl = (ln * 5 // 8) & ~1          # DVE share (even)
                h = dl // 2
                nc.vector.scalar_tensor_tensor(
                    out=scr_d[:, :h], in0=XL[0:64, a:a + h], scalar=1.0,
                    in1=XL[0:64, a + h:a + dl], op0=Alu.mult, op1=Alu.add,
                    accum_out=PARTS[:, c:c + 1])
                nc.scalar.activation(
                    out=scr_a[:, :ln - dl], in_=XL[0:64, a + dl:a + ln],
                    func=Act.Identity, bias=0.0, scale=1.0,
                    accum_out=PARTS[:, 5 + c:6 + c])
                cv0 = XL[0:64, a:a + ln].rearrange("p (r w) -> p r w", w=WP)
                nc.vector.tensor_reduce(out=COLP[:, c:c + 1],
                                        in_=cv0[:, :, 0:1], axis=Ax.XY, op=Alu.add)
                nc.vector.tensor_reduce(out=COLP[:, 5 + c:6 + c],
                                        in_=cv0[:, :, 127:128], axis=Ax.XY, op=Alu.add)
                if c == 0:
                    # rows 0-8 chunk: row-0 sum + top corners
                    nc.vector.tensor_reduce(out=M10[:, 1:2],
                                            in_=XL[0:64, 132:132 + W],
                                            axis=Ax.X, op=Alu.add)
                    nc.vector.tensor_copy(
                        out=M10[:, 5:7].rearrange("p (a b) -> p a b", b=1),
                        in_=XL[0:64, 132:132 + 254].rearrange(
                            "p (a b) -> p a b", b=127)[:, :, 0:1])
                if c == 1:
                    # rows 96-128 chunk: row-127 sum + bottom corners
                    nc.vector.tensor_reduce(out=M10[:, 2:3],
                                            in_=XL[0:64, 16642:16642 + W],
                                            axis=Ax.X, op=Alu.add)
                    nc.vector.tensor_copy(
                        out=M10[:, 7:9].rearrange("p (a b) -> p a b", b=1),
                        in_=XL[0:64, 16642:16642 + 254].rearrange(
                            "p (a b) -> p a b", b=127)[:, :, 0:1])
                # keep the PE clock ramped: dummy matmul gated on this chunk
                nc.tensor.matmul(wpsum[:, 0:512], zrow[0:64, :],
                                 XL[0:64, a:a + 512], start=True, stop=True)

            # copy conv bias (bf16, embedded in wbs) to fp32
            nc.vector.tensor_copy(out=convb,
                                  in_=wb_sb[0:64, K * 384 + 40:K * 384 + 41])

            # fold partials: col sums and basis column 0
            nc.vector.tensor_reduce(out=M10[:, 3:4], in_=COLP[:, 0:5],
                                    axis=Ax.X, op=Alu.add)
            nc.vector.tensor_reduce(out=M10[:, 4:5], in_=COLP[:, 5:10],
                                    axis=Ax.X, op=Alu.add)
            nc.vector.tensor_reduce(out=M10[:, 0:1], in_=PARTS, axis=Ax.X, op=Alu.add)

            # per-channel coefficient contraction: G[c,k] = sum_b M10[c,b]*CW2[c,b,k]
            for k in range(K):
                nc.vector.scalar_tensor_tensor(
                    out=scr_g[:, 0:10], in0=M10[:, :], scalar=1.0,
                    in1=cw2v[:, :, k], op0=Alu.mult, op1=Alu.mult,
                    accum_out=G[:, k:k + 1])

            # logits broadcast; exp + sum in one ACT op; normalize on DVE
            nc.tensor.matmul(psum_b, onesall, G, start=True, stop=True)
            nc.scalar.activation(out=att_sb, in_=psum_b, func=Act.Exp,
                                 accum_out=rs_sum)
            # late upper-copy chunks issued from the scalar engine only after
            # its attention work (a dma_start blocks the engine if the hwdge
            # ring is full; by now the ring has drained)
            for a, ln in U_SCAL:
                nc.scalar.dma_start(out=XL[64:128, 2 + a:2 + a + ln],
                                    in_=xin[:, a:a + ln])
            nc.vector.reciprocal(out=rs_inv, in_=rs_sum)
            nc.vector.tensor_scalar_mul(out=attn, in0=att_sb, scalar1=rs_inv)

            # --- weight mixing: mw = sum_k attn_k * bank'_k, one group per
            # conv window so the first matmul unblocks after group 0 ---
            wv = wb_sb[:, 0:K * 384].rearrange("p (k c) -> p k c", k=K)
            mwf = mw.rearrange("p a b -> p (a b)")
            mbf = mwb.rearrange("p a b -> p (a b)")
            for g in range(3):
                sl = slice(g * 128, g * 128 + 128)
                nc.vector.tensor_scalar_mul(
                    out=mwf[:, sl], in0=wv[:, 0, sl], scalar1=attn[:, 0:1])
                for k in range(1, K):
                    tgt = mbf if k == K - 1 else mwf
                    nc.vector.scalar_tensor_tensor(
                        out=tgt[:, sl], in0=wv[:, k, sl],
                        scalar=attn[:, k:k + 1], in1=mwf[:, sl],
                        op0=Alu.mult, op1=Alu.add)

            # --- main conv: 43 dual-plane PSUM tiles x 3 matmuls ---
            # Eviction: ACT copies plane B (PSUM parts 64-127) into bplane_j
            # aligned to output rows (bplane_j[q] = B-contribution of out row
            # 3j+q, i.e. B_j rows 0,1 -> bplane_j[1:3], B_j row 2 ->
            # bplane_{j+1}[0]); DVE then does ONE STT per tile:
            # stg = (A-plane + convb) + bplane.
            # tile -> out-DMA group map: groups of 6 tiles, smaller trailing
            # groups so the final DMA (and the kernel tail) is short
            grp_of = {}
            gstart = {}
            gg = 0
            jj = 0
            for size in (6, 6, 6, 6, 6, 6, 3, 3, 1):
                gstart[gg] = jj
                for _ in range(size):
                    grp_of[jj] = gg
                    jj += 1
                gg += 1
            ntiles = (H + RPT - 1) // RPT
            stg = None
            nc.vector.memset(bplane[:, 0:WP], 0.0)
            for j in range(ntiles):
                a = RPT * j
                pt = PS.tile([128, F3], f32, tag="cps", name=f"cps{j}")
                for w in range(3):
                    o = 131 + WP * a + (w - 1)
                    nc.tensor.matmul(pt, mwb[:, w, :], XL[:, o:o + F3],
                                     start=(w == 0), stop=(w == 2))
                nrows = min(RPT, H - a)
                g = grp_of[j]
                gi = j - gstart[g]
                if gi == 0:
                    stg = STG.tile([64, 6 * RPT * WP], bf16,
                                   tag="stg", name=f"stg{g}")
                dst = stg[:, gi * RPT * WP: gi * RPT * WP + nrows * WP]
                # ACT: B_j rows 0..2 -> bplane rows 3j+1..3j+3 (contiguous)
                nc.scalar.activation(
                    out=bplane[:, (RPT * j + 1) * WP:(RPT * j + 1 + nrows) * WP],
                    in_=pt[64:128, 0:nrows * WP], func=Act.Identity,
                    bias=0.0, scale=1.0)
                # DVE: stg = (A + convb) + bplane[3j:3j+nrows] (contiguous)
                nc.vector.scalar_tensor_tensor(
                    out=dst, in0=pt[0:64, 0:nrows * WP], scalar=convb,
                    in1=bplane[:, RPT * j * WP:(RPT * j + nrows) * WP],
                    op0=Alu.add, op1=Alu.add)
                if j == ntiles - 1 or grp_of[j + 1] != g:
                    g0row = gstart[g] * RPT
                    grows = min(H - g0row, (j + 1 - gstart[g]) * RPT)
                    src = stg[:, :grows * WP].rearrange("p (r w) -> p r w", w=WP)
                    eng = nc.sync if g % 2 == 0 else nc.scalar
                    eng.dma_start(out=outT[:, g0row:g0row + grows, :], in_=src)

    nc.compile()
    return nc


def _get_nc():
    if "nc" not in _NC_CACHE:
        _NC_CACHE["nc"] = _build_nc()
    return _NC_CACHE["nc"]


def _prep_inputs(x, weight, conv_w, conv_b, net0_w, net0_b, net1_w, net1_b,
                 net2_w, net2_b):
    import ml_dtypes
    cw2 = _make_cw2(np.asarray(net0_w, np.float32), np.asarray(net0_b, np.float32),
                    np.asarray(net1_w, np.float32), np.asarray(net1_b, np.float32),
                    np.asarray(net2_w, np.float32), np.asarray(net2_b, np.float32))
    wsum = np.asarray(weight, np.float32) + np.asarray(conv_w, np.float32)[None]
    banks = np.stack([_make_bank3(wsum[k]) for k in range(K)])  # (K,128,3,128)
    bf = banks.reshape(K, 128, 384)
    wbs = np.zeros((128, K * 384 + 48), np.float32)
    for k in range(K):
        wbs[:, k * 384:(k + 1) * 384] = bf[k]
    wbs[0:64, K * 384:K * 384 + 40] = cw2.reshape(C, 40)
    wbs[0:64, K * 384 + 40] = np.asarray(conv_b, np.float32)
    wbs = np.ascontiguousarray(wbs).astype(ml_dtypes.bfloat16)
    x = np.asarray(x, np.float32)
    xp = np.zeros((N, C, H, WP), np.float32)
    xp[:, :, :, :W] = x
    xs = xp.astype(ml_dtypes.bfloat16)
    in_maps = []
    for n in range(N):
        in_maps.append({
            "xin": np.ascontiguousarray(xs[n].reshape(C, HWELEM)),
            "wbs": wbs,
        })
    return in_maps


def _run(inputs, trace=False, **kw):
    from concourse.bass_utils import run_bass_kernel_spmd
    nc = _get_nc()
    in_maps = _prep_inputs(**inputs)
    return run_bass_kernel_spmd(nc, in_maps, core_ids=list(range(N)), trace=trace, **kw)


def kernel(**inputs):
    res = _run(inputs)
    out = np.stack([np.asarray(res.results[n]["out"]) for n in range(N)]).astype(np.float32)
    return np.ascontiguousarray(out[:, :, :, 1:1 + W])
